# revision 49
# baseline (speedup 1.0000x reference)
"""GCN (2-layer + BN + global mean pool + sigmoid readout) on 8 TRN2 NeuronCores.

Strategy (see spec sharding_hint): destinations (nodes) sharded across the 8
cores; each core aggregates messages for its node shard.  Per layer:

  y = dinv * (X @ W)            (node-major, bf16, exchanged via AllGather)
  agg[c] = dinv[c] * (sum_{e: col_e==c} y[row_e]  +  y[c])   (self loop direct)
  h = relu(BN(agg))

Perf structure (HW-measured: the SWDGE indexed gather is ~95% of runtime,
~4.7ns/descriptor across 4 queues; everything else hides under it):
 - y is exchanged as TWO row-range tables via Shared-output AllGathers (the
   fast collective path), so the A-table gather/segment pass overlaps the
   B-table AllGather.
 - One dma_gather piece per few chunks of each (window, table) run; each
   core's padding is a trailing run of idx=-1 slots the ucode skips, with a
   shared num_idxs_reg equal to the cross-core max real count (dummy idx=0
   slots make the count identical on every core).
 - Self loops never gather: their y stays SBUF-resident node-major and is
   added with one ACT mul + DVE add per tile.
 - The segment-sum is a TensorE matmul of each gathered 128-edge chunk
   against a one-hot S matrix built on DVE (is_equal vs iota), accumulated in
   PSUM per 512-destination block; BN stats transposes run inside pass B so
   they overlap the gather DMA.
The instruction stream is identical on all 8 cores (SPMD); all per-core
variation lives in the input data (indices, selection metadata, padding).
"""

import numpy as np
import ml_dtypes

import concourse.bacc as bacc
import concourse.bass as bass
import concourse.tile as tile
from concourse import mybir
from concourse.bass_utils import run_bass_kernel_spmd

BF16 = ml_dtypes.bfloat16
P = 128          # partitions / chunk size
WIN = 64         # dest window width (S matrix width)
BLOCK_WINS = 8   # windows per PSUM block (8*64 = 512 dests)
PIECE_CHUNKS = 99  # chunks per gather piece; whole runs (~640 desc) pipeline best
EPS = 1e-5


class Dims:
    def __init__(self, N=50000, E=800000, F=96, H=128, G=64, C=50,
                 ncores=8):
        assert N % ncores == 0
        self.N, self.E, self.F, self.H = N, E, F, H
        self.G, self.C = G, C
        self.ncores = ncores
        self.shard = N // ncores
        self.ntile = ceil_div(self.shard, P)  # node tiles per shard
        self.shard_pad = self.ntile * P       # padded shard rows in y table
        # local row-range split: table A = rows [0, S0), table B = [S0, pad).
        # Both 512-aligned (mt-group granularity); each table's all-gathered
        # row count stays within int16 index range for dma_gather.
        self.S0 = 3072
        self.S1 = self.shard_pad - self.S0
        self.npadA = self.S0 * ncores         # 24576 < 32768
        self.npadB = self.S1 * ncores         # 25600 < 32768
        assert self.npadA < 32768 and self.npadB < 32768
        self.nwin = ceil_div(self.shard, WIN)
        self.nblk = ceil_div(self.nwin, BLOCK_WINS)


def ceil_div(a, b):
    return (a + b - 1) // b


# ----------------------------------------------------------------------------
# Host planning: pure index/graph preprocessing (functions of edge_index/batch)
# ----------------------------------------------------------------------------

class Plan:
    pass


def make_plan(d: Dims, edge_index: np.ndarray, batch: np.ndarray) -> Plan:
    pl = Plan()
    N, E = d.N, d.E
    # self-loops are NOT routed through the gather: each core adds
    # dinv^2 * y for its own nodes directly on-chip. deg still counts them.
    rows = edge_index[0].astype(np.int64)
    cols = edge_index[1].astype(np.int64)
    deg = (np.bincount(cols, minlength=N) + 1).astype(np.float64)
    dinv = (1.0 / np.sqrt(np.maximum(deg, 1.0))).astype(np.float32)

    core_of = cols // d.shard
    # remap source node id to its position in table A or B of the split
    # all-gathered y tables (half 0 = table A, half 1 = table B); the stored
    # index is already table-local.
    k_src = rows // d.shard
    r_loc = rows % d.shard
    lo_all = r_loc < d.S0
    pid = np.where(lo_all, k_src * d.S0 + r_loc,
                   k_src * d.S1 + (r_loc - d.S0))
    # Per (core, window, half) edge lists, edges sorted by local dest.
    per_core = []
    for k in range(d.ncores):
        m = core_of == k
        r = pid[m]
        c = cols[m] - k * d.shard
        lo = lo_all[m]
        order = np.argsort(c, kind="stable")
        r, c, lo = r[order], c[order], lo[order]
        w = c // WIN
        lists = {}
        # bucket by (window, half) preserving dest order
        for half_id, mask in ((0, lo), (1, ~lo)):
            rw, cw, ww = r[mask], c[mask], w[mask]
            # indices where window changes
            for wi in range(d.nwin):
                sel = ww == wi
                lists[(wi, half_id)] = (rw[sel], cw[sel])
        per_core.append(lists)

    # Shared chunk schedule: R[w][half] = max over cores of ceil(count/128)
    R = np.zeros((d.nwin, 2), dtype=np.int64)
    for k in range(d.ncores):
        for (wi, hf), (rw, cw) in per_core[k].items():
            R[wi, hf] = max(R[wi, hf], ceil_div(len(rw), P))
    R = np.maximum(R, 0)
    # every window must be initialized in PSUM: ensure at least one chunk
    for wi in range(d.nwin):
        if R[wi].sum() == 0:
            R[wi, 0] = 1

    # Build the chunk stream: per block: [lo chunks (w asc)] ++ [hi chunks]
    stream = []          # list of (window, half) per chunk position
    groups = []          # (block, half, chunk_start, chunk_count)
    for b in range(d.nblk):
        wlo = b * BLOCK_WINS
        whi = min(wlo + BLOCK_WINS, d.nwin)
        for hf in (0, 1):
            g0 = len(stream)
            for wi in range(wlo, whi):
                for _ in range(R[wi, hf]):
                    stream.append((wi, hf))
            groups.append((b, hf, g0, len(stream) - g0))
    C_grid = len(stream)
    tot_slots = C_grid * P

    # start/stop flags: matmul start=True zeroes the ENTIRE 2KB PSUM strip of
    # its output partitions, so exactly one start per (block, parity strip) --
    # the first chunk in stream order touching that strip; stop on the last.
    # all-accumulate scheme: the block PSUM tile is DVE-memset to zero, every
    # matmul uses start=False (accumulate). A start=True would zero the whole
    # 2KB PSUM strip of its partitions, wiping sibling windows in the bank.
    start_flag = np.zeros(C_grid, dtype=bool)
    stop_flag = np.zeros(C_grid, dtype=bool)

    # Fill per-core slot data. Pad slots keep idx=-1: the gather ucode skips
    # trailing negative indices, so per-(window,half)-run gather instructions
    # transfer only each core's real edges (padding varies per core).
    idx_all = np.full((d.ncores, tot_slots), -1, dtype=np.int16)
    A_all = np.full((d.ncores, C_grid, P), 300.0, dtype=np.float32)
    # chunk positions per (window, half) in stream order:
    pos_of = {}
    for pos, key in enumerate(stream):
        pos_of.setdefault(key, []).append(pos)
    # shared per-run valid count V = max over cores of real edges in the run.
    # num_idxs_reg must equal the count of non-negative indices and is a
    # shared immediate, so every core pads its run with dummy (idx=0, no
    # dest) slots up to V; slots beyond V keep idx=-1 and are skipped.
    V = np.zeros((d.nwin, 2), dtype=np.int64)
    for k in range(d.ncores):
        for (wi, hf), (rw, cw) in per_core[k].items():
            V[wi, hf] = max(V[wi, hf], len(rw))
    for wi in range(d.nwin):
        if V[wi].sum() == 0:
            V[wi, 0] = 1      # matches the R fixup: run exists, 1 dummy slot
    for k in range(d.ncores):
        for (wi, hf), (rw, cw) in per_core[k].items():
            n = len(rw)
            positions = pos_of.get((wi, hf), [])
            if not positions:
                assert n == 0
                continue
            assert n <= len(positions) * P
            vals = rw          # already table-local (split tables A/B)
            crel = cw - wi * WIN
            for j, pos in enumerate(positions):
                a, bnd = j * P, min((j + 1) * P, n)
                if a < n:
                    cnt = bnd - a
                    idx_all[k, pos * P: pos * P + cnt] = \
                        vals[a:bnd].astype(np.int16)
                    A_all[k, pos, :cnt] = crel[a:bnd].astype(np.float32)
            # dummy-valid padding up to the shared count V
            run0 = positions[0] * P
            idx_all[k, run0 + n: run0 + V[wi, hf]] = 0

    # wrap idx to the [128, tot_slots//16] layout dma_gather wants:
    # slot i -> [16*c + i%16, i//16] for every q7 core c
    S16 = tot_slots // 16
    idx_wrapped = np.zeros((d.ncores, P, S16), dtype=np.int16)
    for k in range(d.ncores):
        w16 = idx_all[k].reshape(S16, 16).T  # [16, S16]
        idx_wrapped[k] = np.tile(w16, (8, 1))

    # A matrix in [128 partitions=slot%128, C_grid] layout
    A_pt = np.transpose(A_all, (0, 2, 1)).astype(BF16)  # [cores, 128, C_grid]

    # per-core node-major helper arrays
    dinv_pt = np.zeros((d.ncores, P, d.ntile), dtype=np.float32)
    pool_pt = np.zeros((d.ncores, P, d.ntile, d.G), dtype=np.float32)
    for k in range(d.ncores):
        base = k * d.shard
        for t in range(d.ntile):
            for p in range(P):
                n0 = t * P + p
                if n0 < d.shard:
                    dinv_pt[k, p, t] = dinv[base + n0]
                    pool_pt[k, p, t, batch[base + n0]] = 1.0

    cnts = np.bincount(batch, minlength=d.G).astype(np.float32)
    inv_cnt = (1.0 / np.maximum(cnts, 1.0)).reshape(d.G, 1)

    pl.R, pl.stream, pl.groups, pl.V = R, stream, groups, V
    pl.C_grid, pl.tot_slots = C_grid, tot_slots
    pl.start_flag, pl.stop_flag = start_flag, stop_flag
    pl.idx_wrapped, pl.A_pt = idx_wrapped, A_pt
    pl.dinv_pt, pl.pool_pt, pl.inv_cnt = dinv_pt, pool_pt.reshape(d.ncores, P, -1), inv_cnt
    pl.max_lo_chunks = max(g[3] for g in groups if g[1] == 0)
    pl.max_hi_chunks = max(g[3] for g in groups if g[1] == 1)
    blk_tot = {}
    for b, hf, g0, gc in groups:
        blk_tot[b] = blk_tot.get(b, 0) + gc
    pl.max_blk_chunks = max(blk_tot.values())
    return pl


# ----------------------------------------------------------------------------
# Bass program
# ----------------------------------------------------------------------------

def build_program(d: Dims, pl: Plan, debug=False, repeat=1, ablate=()):
    nc = bacc.Bacc("TRN2", target_bir_lowering=False, debug=False,
                   num_devices=d.ncores, num_swdge_queues=4)
    f32, bf16, i16 = mybir.dt.float32, mybir.dt.bfloat16, mybir.dt.int16

    def din(name, shape, dt=f32):
        return nc.dram_tensor(name, shape, dt, kind="ExternalInput").ap()

    xt = din("xt", [d.F, d.shard])
    W1 = din("W1", [d.F, d.H])
    W2 = din("W2", [d.H, d.H], bf16)
    Wc = din("Wc", [d.H, d.C])
    g1 = din("g1", [d.H, 1])
    be1 = din("be1", [d.H, 1])
    g2 = din("g2", [d.H, 1])
    be2 = din("be2", [d.H, 1])
    idx_d = din("idx", [P, pl.tot_slots // 16], i16)
    if "gathpair" in ablate or "gathhalf" in ablate:
        idxh_d = din("idxh", [P, pl.tot_slots // 32], i16)
    A_d = din("A", [P, pl.C_grid], bf16)
    dinv_d = din("dinv_pt", [P, d.ntile])
    pool_d = din("pool_pt", [P, d.ntile * d.G])
    invc_d = din("inv_cnt", [d.G, 1])
    bcr_d = din("bc_rep", [d.G, d.C])
    iota_d = din("iota", [P, WIN], bf16)
    ident_d = din("ident", [P, P])
    out_d = nc.dram_tensor("out", [d.G, d.C], f32, kind="ExternalOutput").ap()
    if debug:
        dbg_agg = nc.dram_tensor("dbg_agg", [P, d.ntile * d.H], f32,
                                 kind="ExternalOutput").ap()
        dbg_h = nc.dram_tensor("dbg_h", [d.H, d.ntile * P], f32,
                               kind="ExternalOutput").ap()
        dbg_y = nc.dram_tensor("dbg_y", [d.npadA + d.npadB, d.H], f32,
                               kind="ExternalOutput").ap()

    rg = [list(range(d.ncores))]

    with tile.TileContext(nc) as tc:
        with (
            tc.tile_pool(name="const", bufs=1) as cpool,
            tc.tile_pool(name="work", bufs=2) as wpool,
            tc.tile_pool(name="glo", bufs=4) as gpool_lo,
            tc.tile_pool(name="ghi", bufs=4) as gpool_hi,
            tc.tile_pool(name="spool", bufs=2) as spool,
            tc.tile_pool(name="big", bufs=1) as bigpool,
            tc.tile_pool(name="pseg", bufs=3, space="PSUM") as pseg,
            tc.tile_pool(name="pmm", bufs=2, space="PSUM") as pmm,
            tc.tile_pool(name="ptr", bufs=3, space="PSUM") as ptr,
            tc.tile_pool(name="dram", bufs=1, space="DRAM") as dpool,
            tc.tile_pool(name="dram_y", bufs=2, space="DRAM") as ypool,
            tc.tile_pool(name="dram_so", bufs=2, space="DRAM") as sopool,
        ):
            # ---- load constants ----
            def cload(ap, shape, dt=f32, name=None):
                t = cpool.tile(shape, dt, tag=name)
                nc.sync.dma_start(out=t[:], in_=ap)
                return t

            W1_s = cload(W1[:], [d.F, d.H], name="W1")
            W2_s = cload(W2[:], [d.H, d.H], bf16, name="W2")
            Wc_s = cload(Wc[:], [d.H, d.C], name="Wc")
            g1_s = cload(g1[:], [d.H, 1], name="g1")
            be1_s = cload(be1[:], [d.H, 1], name="be1")
            g2_s = cload(g2[:], [d.H, 1], name="g2")
            be2_s = cload(be2[:], [d.H, 1], name="be2")
            idx_s = cload(idx_d[:], [P, pl.tot_slots // 16], i16, name="idx")
            if "gathpair" in ablate or "gathhalf" in ablate:
                idxh_s = cload(idxh_d[:], [P, pl.tot_slots // 32], i16,
                               name="idxh")
            A_s = cload(A_d[:], [P, pl.C_grid], bf16, name="A")
            dinv_s = cload(dinv_d[:], [P, d.ntile], name="dinv")
            pool_s = cload(pool_d[:], [P, d.ntile * d.G], name="pool")
            invc_s = cload(invc_d[:], [d.G, 1], name="invc")
            bcr_s = cload(bcr_d[:], [d.G, d.C], name="bcr")
            iota_s = cload(iota_d[:], [P, WIN], bf16, name="iota")
            ident_s = cload(ident_d[:], [P, P], name="ident")

            # pool matrix as bf16 for matmul
            pool_bf = cpool.tile([P, d.ntile * d.G], bf16, tag="poolbf")
            nc.vector.tensor_copy(out=pool_bf[:], in_=pool_s[:])

            eps_s = cpool.tile([d.H, 1], f32, tag="eps")
            nc.vector.memset(eps_s[:], EPS)
            ident_bf = cpool.tile([P, P], bf16, tag="identbf")
            nc.vector.tensor_copy(out=ident_bf[:], in_=ident_s[:])

            # ---- internal DRAM for collectives ----
            adsp = "Local" if "nosharedout" in ablate else "Shared"
            y_own = dpool.tile([d.shard_pad, d.H], bf16)
            stats_in = dpool.tile([d.H, 2], f32)
            pool_in = dpool.tile([d.G, d.H], f32)
            pool_out = dpool.tile([d.G, d.H], f32, addr_space=adsp)

            h_fm = None  # feature-major relu'd activations [H, shard]
            gq = [0]
            qload = [0, 0, 0, 0]  # greedy per-queue descriptor balance

            # one-time zero of the gather pool buffers: slots skipped by the
            # ucode (trailing idx=-1) leave SBUF untouched, and S=0 only
            # protects against finite garbage (0*NaN would poison PSUM)
            for gp, mg, tg in ((gpool_lo, pl.max_lo_chunks, "g0"),
                               (gpool_hi, pl.max_hi_chunks, "g1")):
                for _ in range(4):  # must touch every pool buffer
                    zt = gp.tile([P, mg, d.H], bf16, tag=tg, name=f"z{tg}")
                    nc.vector.memset(zt[:], 0.0)

            for rep in range(repeat):
              for layer in range(2):
                y_fullA = ypool.tile([d.npadA, d.H], bf16, addr_space=adsp,
                                     name=f"y_fullA_r{rep}l{layer}")
                y_fullB = ypool.tile([d.npadB, d.H], bf16, addr_space=adsp,
                                     name=f"y_fullB_r{rep}l{layer}")
                stats_out = sopool.tile([d.H, 2], f32, addr_space=adsp,
                                        name=f"stats_out_r{rep}l{layer}")
                # ---------- y = dinv * (X @ W)  (own shard, node-major) ----
                # staged: compute rows [0,S0) then AllGather table A, then
                # rows [S0,shard) and AllGather table B, so the A-pass
                # gather/segmm below overlaps the B AllGather. The node-major
                # y stays resident in SBUF for the self-loop contribution.
                y_sb = bigpool.tile([P, d.ntile, d.H], bf16, tag="y_sb")
                n_mt = ceil_div(d.shard, 512)
                for phase in (0, 1):
                    mtr = (range(0, d.S0 // 512) if phase == 0
                           else range(d.S0 // 512, n_mt))
                    for mt in mtr:
                        c0 = mt * 512
                        cw = min(512, d.shard - c0)
                        nst = ceil_div(cw, P)
                        if layer == 0:
                            rhs_t = wpool.tile([d.F, 512], f32, tag="xt_t")
                            nc.sync.dma_start(out=rhs_t[:, :cw],
                                              in_=xt[:, c0:c0 + cw])
                            lhsT, rhs_ap = W1_s[:, :], rhs_t[:, :cw]
                        else:
                            lhsT, rhs_ap = W2_s[:, :], h_fm[:, c0:c0 + cw]
                        xw_ps = pmm.tile([d.H, 512], f32, tag="xw")
                        nc.tensor.matmul(out=xw_ps[:, :cw], lhsT=lhsT,
                                         rhs=rhs_ap, start=True, stop=True)
                        xw_sb = wpool.tile([d.H, 512], f32, tag="xw_sb")
                        nc.scalar.copy(out=xw_sb[:, :cw], in_=xw_ps[:, :cw])
                        # transpose 128-node subtiles; dinv scale in ACT evac
                        for st in range(nst):
                            t_global = mt * 4 + st
                            n0 = st * P
                            nw = min(P, cw - n0)
                            tr_ps = ptr.tile([P, d.H], f32, tag="ptr")
                            nc.tensor.transpose(out=tr_ps[:nw, :],
                                                in_=xw_sb[:, n0:n0 + nw],
                                                identity=ident_s[:])
                            nc.scalar.mul(out=y_sb[:nw, t_global, :],
                                          in_=tr_ps[:nw, :],
                                          mul=dinv_s[:nw,
                                                     t_global:t_global + 1])
                        nc.sync.dma_start(
                            out=y_own[c0:c0 + nst * P, :].rearrange(
                                "(t p) f -> p t f", p=P),
                            in_=y_sb[:, mt * 4:mt * 4 + nst, :])
                    y_in = (y_own[0:d.S0, :] if phase == 0
                            else y_own[d.S0:d.shard_pad, :])
                    y_out = y_fullA if phase == 0 else y_fullB
                    if "nogather_collective" in ablate:
                        nc.sync.dma_start(
                            out=y_out[0:(d.S0 if phase == 0 else d.S1), :],
                            in_=y_in)
                    else:
                        nc.gpsimd.collective_compute(
                            "AllGather", mybir.AluOpType.bypass,
                            replica_groups=rg,
                            ins=[y_in.opt()], outs=[y_out.opt()])

                # ---------- gather + segment matmul: pass A, then pass B ----
                probe = ("gathpair" in ablate) or ("gathhalf" in ablate)
                agg_dm = bigpool.tile([P, d.ntile, d.H], f32, tag="agg_dm")
                # feature-major bf16 copy of agg + BN stats, produced
                # incrementally during pass B (overlaps gather DMA)
                agg_fm = bigpool.tile([d.H, d.ntile * P], bf16, tag="agg_fm")
                s1p = wpool.tile([d.H, d.ntile], f32, tag="s1p")
                s2p = wpool.tile([d.H, d.ntile], f32, tag="s2p")
                scratch = wpool.tile([d.H, P], f32, tag="scr")
                if d.shard % P:
                    nc.vector.memset(agg_dm[:, d.ntile - 1, :], 0.0)
                for hf in (0, 1):
                    gpool = gpool_lo if hf == 0 else gpool_hi
                    ysrc = (y_fullA if hf == 0 else y_fullB)[:, :]
                    mgc = pl.max_lo_chunks if hf == 0 else pl.max_hi_chunks
                    for b in range(d.nblk):
                        wlo = b * BLOCK_WINS
                        whi = min(wlo + BLOCK_WINS, d.nwin)
                        _, _, g0, gcnt = pl.groups[2 * b + hf]
                        empty_b = hf == 1 and gcnt == 0
                        blk_ps = None
                        if not empty_b:
                            blk_ps = pseg.tile([P, 4 * d.H], f32, tag="seg")
                            nc.vector.memset(blk_ps[:], 0.0)
                        gt = None
                        if (gcnt and not empty_b
                                and "nodmagather" not in ablate and not probe):
                            gt = gpool.tile([P, mgc, d.H], bf16, tag=f"g{hf}")
                            if "contiggather" in ablate:
                                nc.sync.dma_start(
                                    out=gt[:, :gcnt, :],
                                    in_=ysrc[0:gcnt * P, :].rearrange(
                                        "(s p) f -> p s f", p=P))
                            else:
                                # one gather per window run: each core's
                                # padding is a trailing run of idx=-1 slots,
                                # which the ucode skips (no transfer)
                                roff = 0
                                for wi in range(wlo, whi):
                                    rc = int(pl.R[wi, hf])
                                    if rc == 0:
                                        continue
                                    V_run = (rc * P if PAD0
                                             else int(pl.V[wi, hf]))
                                    # split the run into small pieces across
                                    # queues; valid slots are a prefix of the
                                    # run, so each piece's reg count is exact
                                    # and empty pieces are skipped entirely
                                    pc = 0
                                    while pc < rc:
                                        pcw = min(PIECE_CHUNKS, rc - pc)
                                        reg = max(0, min(V_run - pc * P,
                                                         pcw * P))
                                        if reg > 0:
                                            ns_pp = pcw * P
                                            s0 = (g0 + roff + pc) * P
                                            qn = gq[0] % 4
                                            nc.gpsimd.dma_gather(
                                                out_ap=gt[:, roff + pc:
                                                          roff + pc + pcw, :],
                                                in_ap=ysrc,
                                                idxs_ap=idx_s[
                                                    :, s0 // 16:
                                                    (s0 + ns_pp) // 16],
                                                num_idxs=ns_pp,
                                                num_idxs_reg=reg,
                                                elem_size=d.H,
                                                single_packet=False,
                                                queue_num=qn,
                                            )
                                            gq[0] += 1
                                        pc += pcw
                                    roff += rc
                                assert roff == gcnt
                        elif gcnt and probe:
                            # timing probes: same bytes/half bytes with half
                            # the descriptors; gathered data unused (implies
                            # no segmm matmuls for this pass)
                            pair = "gathpair" in ablate
                            esz = 2 * d.H if pair else d.H
                            gtp = gpool.tile([P, ceil_div(mgc, 2), esz], bf16,
                                             tag=f"gp{hf}")
                            ns_p = gcnt * P
                            ns_h = ns_p // 2
                            ysrc_p = (ysrc.rearrange("(a two) f -> a (two f)",
                                                     two=2) if pair else ysrc)
                            nc.gpsimd.dma_gather(
                                out_ap=gtp[:, 0:ceil_div(ns_h, P), :],
                                in_ap=ysrc_p,
                                idxs_ap=idxh_s[:, (g0 * P) // 32:
                                               (g0 * P) // 32 + ns_h // 16],
                                num_idxs=ns_h,
                                num_idxs_reg=ns_h,
                                elem_size=esz,
                                single_packet=False,
                                queue_num=gq[0] % 4,
                            )
                            gq[0] += 1
                        if gcnt and gt is not None:
                            S_t = spool.tile(
                                [P, max(pl.max_lo_chunks, pl.max_hi_chunks),
                                 WIN], bf16, tag="S")
                            a_b = A_s[:, g0:g0 + gcnt].unsqueeze(2) \
                                .broadcast_to([P, gcnt, WIN])
                            i_b = iota_s[:].unsqueeze(1) \
                                .broadcast_to([P, gcnt, WIN])
                            nc.vector.tensor_tensor(out=S_t[:, :gcnt, :],
                                                    in0=a_b, in1=i_b,
                                                    op=mybir.AluOpType.is_equal)
                            for pos in (() if "nosegmm" in ablate
                                        else range(g0, g0 + gcnt)):
                                wi, _hx = pl.stream[pos]
                                lc = pos - g0
                                w_in_b = wi - wlo
                                wpp = P // WIN
                                pof = WIN * (w_in_b % wpp)
                                fof = d.H * (w_in_b // wpp)
                                nc.tensor.matmul(
                                    out=blk_ps[pof:pof + WIN, fof:fof + d.H],
                                    lhsT=S_t[:, lc, :],
                                    rhs=gt[:, lc, :],
                                    start=False, stop=False,
                                    skip_group_check=True,
                                )
                        # evacuate: dest-major agg with dinv scaling; pass B
                        # accumulates on top of pass A, then immediately
                        # transposes each finished tile for BN stats + the
                        # feature-major agg copy (overlaps later gathers)
                        for w4 in range(ceil_div((whi - wlo) * WIN, P)):
                            t_global = (BLOCK_WINS * WIN // P) * b + w4
                            nw = min(P, d.shard - t_global * P)
                            if hf == 0:
                                nc.scalar.mul(
                                    out=agg_dm[:nw, t_global, :],
                                    in_=blk_ps[:nw, w4 * d.H:(w4 + 1) * d.H],
                                    mul=dinv_s[:nw, t_global:t_global + 1])
                                # direct self-loop term: dinv^2 * xw = dinv * y
                                # (y already carries one dinv factor)
                                slt = wpool.tile([P, d.H], f32, tag="slt")
                                nc.scalar.mul(
                                    out=slt[:nw, :],
                                    in_=y_sb[:nw, t_global, :],
                                    mul=dinv_s[:nw, t_global:t_global + 1])
                                nc.vector.tensor_tensor(
                                    out=agg_dm[:nw, t_global, :],
                                    in0=agg_dm[:nw, t_global, :],
                                    in1=slt[:nw, :],
                                    op=mybir.AluOpType.add)
                                continue
                            if not empty_b:
                                evB = wpool.tile([P, d.H], f32, tag="evB")
                                nc.scalar.mul(
                                    out=evB[:nw, :],
                                    in_=blk_ps[:nw, w4 * d.H:(w4 + 1) * d.H],
                                    mul=dinv_s[:nw, t_global:t_global + 1])
                                nc.vector.tensor_tensor(
                                    out=agg_dm[:nw, t_global, :],
                                    in0=agg_dm[:nw, t_global, :],
                                    in1=evB[:nw, :],
                                    op=mybir.AluOpType.add)
                            t = t_global
                            tr_ps = ptr.tile([d.H, P], f32, tag="ptr")
                            nc.tensor.transpose(out=tr_ps[:, :],
                                                in_=agg_dm[:, t, :],
                                                identity=ident_s[:])
                            nc.scalar.activation(
                                out=agg_fm[:, t * P:(t + 1) * P],
                                in_=tr_ps[:],
                                func=mybir.ActivationFunctionType.Copy,
                                accum_out=s1p[:, t:t + 1])
                            nc.scalar.activation(
                                out=scratch[:], in_=tr_ps[:],
                                func=mybir.ActivationFunctionType.Square,
                                accum_out=s2p[:, t:t + 1])

                stats_sb = wpool.tile([d.H, 2], f32, tag="stats")
                nc.vector.tensor_reduce(out=stats_sb[:, 0:1], in_=s1p[:],
                                        axis=mybir.AxisListType.X,
                                        op=mybir.AluOpType.add)
                nc.vector.tensor_reduce(out=stats_sb[:, 1:2], in_=s2p[:],
                                        axis=mybir.AxisListType.X,
                                        op=mybir.AluOpType.add)
                nc.sync.dma_start(out=stats_in[:], in_=stats_sb[:])
                if "nostatsar" in ablate:
                    nc.sync.dma_start(out=stats_out[:], in_=stats_in[:])
                else:
                    nc.gpsimd.collective_compute(
                        "AllReduce", mybir.AluOpType.add, replica_groups=rg,
                        ins=[stats_in.opt()], outs=[stats_out.opt()])
                stats_g = wpool.tile([d.H, 2], f32, tag="statsg")
                nc.sync.dma_start(out=stats_g[:], in_=stats_out[:])
                # mean/var -> scale/bias
                mv = wpool.tile([d.H, 6], f32, tag="mv")
                inv_n = 1.0 / d.N
                nc.vector.tensor_scalar(out=mv[:, 0:1], in0=stats_g[:, 0:1],
                                        scalar1=inv_n, scalar2=None,
                                        op0=mybir.AluOpType.mult)  # mean
                nc.vector.tensor_scalar(out=mv[:, 1:2], in0=stats_g[:, 1:2],
                                        scalar1=inv_n, scalar2=None,
                                        op0=mybir.AluOpType.mult)  # E[x^2]
                nc.vector.tensor_tensor(out=mv[:, 2:3], in0=mv[:, 0:1],
                                        in1=mv[:, 0:1],
                                        op=mybir.AluOpType.mult)   # mean^2
                nc.vector.tensor_tensor(out=mv[:, 2:3], in0=mv[:, 1:2],
                                        in1=mv[:, 2:3],
                                        op=mybir.AluOpType.subtract)  # var
                nc.scalar.activation(out=mv[:, 3:4], in_=mv[:, 2:3],
                                     func=mybir.ActivationFunctionType.Sqrt,
                                     bias=eps_s[:])                # std
                nc.vector.reciprocal(out=mv[:, 4:5], in_=mv[:, 3:4])
                gg = g1_s if layer == 0 else g2_s
                bb = be1_s if layer == 0 else be2_s
                nc.vector.tensor_tensor(out=mv[:, 4:5], in0=mv[:, 4:5],
                                        in1=gg[:], op=mybir.AluOpType.mult)
                # bias = be - mean*scale
                nc.vector.tensor_tensor(out=mv[:, 5:6], in0=mv[:, 0:1],
                                        in1=mv[:, 4:5],
                                        op=mybir.AluOpType.mult)
                nc.vector.tensor_tensor(out=mv[:, 5:6], in0=bb[:],
                                        in1=mv[:, 5:6],
                                        op=mybir.AluOpType.subtract)
                if debug and layer == 0:
                    for tbl, base in ((y_fullA, 0), (y_fullB, d.npadA)):
                        for t in range(tbl.shape[0] // P):
                            dbg_y_bf = wpool.tile([P, d.H], bf16,
                                                  tag="dbgybf")
                            dbg_y_sb = wpool.tile([P, d.H], f32, tag="dbgy")
                            nc.sync.dma_start(
                                out=dbg_y_bf[:],
                                in_=tbl[t * P:(t + 1) * P, :])
                            nc.vector.tensor_copy(out=dbg_y_sb[:],
                                                  in_=dbg_y_bf[:])
                            nc.sync.dma_start(
                                out=dbg_y[base + t * P:base + (t + 1) * P, :],
                                in_=dbg_y_sb[:])
                    nc.sync.dma_start(
                        out=dbg_agg[:],
                        in_=agg_dm[:].rearrange("p t f -> p (t f)"))
                h_fm = bigpool.tile([d.H, d.ntile * P], bf16, tag="h_fm")
                for t in range(d.ntile):
                    nc.scalar.activation(out=h_fm[:, t * P:(t + 1) * P],
                                         in_=agg_fm[:, t * P:(t + 1) * P],
                                         func=mybir.ActivationFunctionType.Relu,
                                         scale=mv[:, 4:5], bias=mv[:, 5:6])

            if debug:
                dbg_h_sb = wpool.tile([d.H, d.ntile * P], f32, tag="dbgh")
                nc.vector.tensor_copy(out=dbg_h_sb[:], in_=h_fm[:])
                nc.sync.dma_start(out=dbg_h[:], in_=dbg_h_sb[:])
            # ---------- pooling ----------
            # node-major h2 tiles via transpose, then matmul with pool matrix
            pool_ps = ptr.tile([d.G, d.H], f32, tag="ptr")
            for t in range(d.ntile):
                tr_ps = ptr.tile([P, d.H], bf16, tag="ptr")
                nc.tensor.transpose(out=tr_ps[:, :],
                                    in_=h_fm[:, t * P:(t + 1) * P],
                                    identity=ident_bf[:])
                h_dm = wpool.tile([P, d.H], bf16, tag="h_dm")
                nc.scalar.copy(out=h_dm[:], in_=tr_ps[:])
                nc.tensor.matmul(
                    out=pool_ps[:, :],
                    lhsT=pool_bf[:, t * d.G:(t + 1) * d.G],
                    rhs=h_dm[:],
                    start=(t == 0), stop=(t == d.ntile - 1))
            pool_sb = wpool.tile([d.G, d.H], f32, tag="poolsb")
            nc.vector.tensor_scalar(out=pool_sb[:], in0=pool_ps[:],
                                    scalar1=invc_s[:], scalar2=None,
                                    op0=mybir.AluOpType.mult)
            nc.sync.dma_start(out=pool_in[:], in_=pool_sb[:])
            if "nopoolar" in ablate:
                nc.sync.dma_start(out=pool_out[:], in_=pool_in[:])
            else:
                nc.gpsimd.collective_compute(
                    "AllReduce", mybir.AluOpType.add, replica_groups=rg,
                    ins=[pool_in.opt()], outs=[pool_out.opt()])
            pooled = wpool.tile([d.G, d.H], f32, tag="pooled")
            nc.sync.dma_start(out=pooled[:], in_=pool_out[:])
            # transpose pooled -> [H, G]
            pooled_t_ps = ptr.tile([d.H, d.G], f32, tag="ptr")
            nc.tensor.transpose(out=pooled_t_ps[:, :], in_=pooled[:],
                                identity=ident_s[:d.G, :d.G])
            pooled_t = wpool.tile([d.H, d.G], f32, tag="pooledtsb")
            nc.scalar.copy(out=pooled_t[:], in_=pooled_t_ps[:])
            out_ps = ptr.tile([d.G, d.C], f32, tag="ptr")
            nc.tensor.matmul(out=out_ps[:], lhsT=pooled_t[:], rhs=Wc_s[:],
                             start=True, stop=True)
            out_sb = wpool.tile([d.G, d.C], f32, tag="outsb")
            nc.vector.tensor_tensor(out=out_sb[:], in0=out_ps[:],
                                    in1=bcr_s[:], op=mybir.AluOpType.add)
            nc.scalar.activation(out=out_sb[:], in_=out_sb[:],
                                 func=mybir.ActivationFunctionType.Sigmoid)
            nc.sync.dma_start(out=out_d[:], in_=out_sb[:])

    nc.compile()
    return nc


# ----------------------------------------------------------------------------
# Entry point
# ----------------------------------------------------------------------------

def make_in_maps(d: Dims, pl: Plan, inputs):
    x = np.asarray(inputs["x"], np.float32)
    W1 = np.asarray(inputs["W1"], np.float32)
    W2 = np.asarray(inputs["W2"], np.float32)
    Wc = np.asarray(inputs["Wc"], np.float32)
    g1 = np.asarray(inputs["g1"], np.float32).reshape(d.H, 1)
    be1 = np.asarray(inputs["be1"], np.float32).reshape(d.H, 1)
    g2 = np.asarray(inputs["g2"], np.float32).reshape(d.H, 1)
    be2 = np.asarray(inputs["be2"], np.float32).reshape(d.H, 1)
    bc = np.asarray(inputs["bc"], np.float32)
    xt = np.ascontiguousarray(x.T)
    iota = np.tile(np.arange(WIN, dtype=np.float32), (P, 1)).astype(BF16)
    ident = np.eye(P, dtype=np.float32)
    bc_rep = np.tile(bc.reshape(1, d.C), (d.G, 1)).astype(np.float32)
    in_maps = []
    for k in range(d.ncores):
        in_maps.append({
            "xt": np.ascontiguousarray(xt[:, k * d.shard:(k + 1) * d.shard]),
            "W1": W1, "W2": W2.astype(BF16), "Wc": Wc,
            "g1": g1, "be1": be1, "g2": g2, "be2": be2,
            "idx": (np.maximum(pl.idx_wrapped[k], 0) if PAD0
                    else pl.idx_wrapped[k]),
            "idxh": np.maximum(pl.idx_wrapped[k][:, :pl.tot_slots // 32], 0)
                    // 2,
            "A": np.ascontiguousarray(pl.A_pt[k]),
            "dinv_pt": pl.dinv_pt[k],
            "pool_pt": pl.pool_pt[k],
            "inv_cnt": pl.inv_cnt,
            "bc_rep": bc_rep,
            "iota": iota,
            "ident": ident,
        })
    return in_maps


PAD0 = False  # True: pad slots gather row 0 instead of being skipped (-1)


def kernel(**inputs) -> np.ndarray:
    d = Dims()
    edge_index = np.asarray(inputs["edge_index"], np.int64)
    batch = np.asarray(inputs["batch"], np.int64)
    pl = make_plan(d, edge_index, batch)
    nc = build_program(d, pl)
    in_maps = make_in_maps(d, pl, inputs)
    res = run_bass_kernel_spmd(nc, in_maps, core_ids=list(range(d.ncores)))
    return np.asarray(res.results[0]["out"], np.float32)



# revision 54
# speedup vs baseline: 1.1037x; 1.1037x over previous
"""GCN (2-layer + BN + global mean pool + sigmoid readout) on 8 TRN2 NeuronCores.

Strategy (see spec sharding_hint): destinations (nodes) sharded across the 8
cores; each core aggregates messages for its node shard.  Per layer:

  y = dinv * (X @ W)            (node-major, bf16, exchanged via AllGather)
  agg[c] = dinv[c] * (sum_{e: col_e==c} y[row_e]  +  y[c])   (self loop direct)
  h = relu(BN(agg))

Perf structure (HW-measured: the SWDGE indexed gather is ~95% of runtime,
~4.7ns/descriptor across 4 queues; everything else hides under it):
 - y is exchanged as TWO row-range tables via Shared-output AllGathers (the
   fast collective path), so the A-table gather/segment pass overlaps the
   B-table AllGather.
 - One dma_gather piece per few chunks of each (window, table) run; each
   core's padding is a trailing run of idx=-1 slots the ucode skips, with a
   shared num_idxs_reg equal to the cross-core max real count (dummy idx=0
   slots make the count identical on every core).
 - Self loops never gather: their y stays SBUF-resident node-major and is
   added with one ACT mul + DVE add per tile.
 - The segment-sum is a TensorE matmul of each gathered 128-edge chunk
   against a one-hot S matrix built on DVE (is_equal vs iota), accumulated in
   PSUM per 512-destination block; BN stats transposes run inside pass B so
   they overlap the gather DMA.
The instruction stream is identical on all 8 cores (SPMD); all per-core
variation lives in the input data (indices, selection metadata, padding).
"""

import numpy as np
import ml_dtypes

import concourse.bacc as bacc
import concourse.bass as bass
import concourse.tile as tile
from concourse import mybir
from concourse.bass_utils import run_bass_kernel_spmd

BF16 = ml_dtypes.bfloat16
P = 128          # partitions / chunk size
WIN = 64         # dest window width (S matrix width)
BLOCK_WINS = 8   # windows per PSUM block (8*64 = 512 dests)
PIECE_CHUNKS = 99  # chunks per gather piece; whole runs (~640 desc) pipeline best
EPS = 1e-5


class Dims:
    def __init__(self, N=50000, E=800000, F=96, H=128, G=64, C=50,
                 ncores=8):
        assert N % ncores == 0
        self.N, self.E, self.F, self.H = N, E, F, H
        self.G, self.C = G, C
        self.ncores = ncores
        self.shard = N // ncores
        self.ntile = ceil_div(self.shard, P)  # node tiles per shard
        self.shard_pad = self.ntile * P       # padded shard rows in y table
        # local row-range split: table A = rows [0, S0), table B = [S0, pad).
        # Both 512-aligned (mt-group granularity); each table's all-gathered
        # row count stays within int16 index range for dma_gather.
        self.S0 = 3072
        self.S1 = self.shard_pad - self.S0
        self.npadA = self.S0 * ncores         # 24576 < 32768
        self.npadB = self.S1 * ncores         # 25600 < 32768
        assert self.npadA < 32768 and self.npadB < 32768
        self.nwin = ceil_div(self.shard, WIN)
        self.nblk = ceil_div(self.nwin, BLOCK_WINS)


def ceil_div(a, b):
    return (a + b - 1) // b


# ----------------------------------------------------------------------------
# Host planning: pure index/graph preprocessing (functions of edge_index/batch)
# ----------------------------------------------------------------------------

class Plan:
    pass


def make_plan(d: Dims, edge_index: np.ndarray, batch: np.ndarray) -> Plan:
    pl = Plan()
    N, E = d.N, d.E
    # self-loops are NOT routed through the gather: each core adds
    # dinv^2 * y for its own nodes directly on-chip. deg still counts them.
    rows = edge_index[0].astype(np.int64)
    cols = edge_index[1].astype(np.int64)
    deg = (np.bincount(cols, minlength=N) + 1).astype(np.float64)
    dinv = (1.0 / np.sqrt(np.maximum(deg, 1.0))).astype(np.float32)

    # Degree-balanced node -> (core, slot) assignment: the shared gather
    # schedule pays max-over-cores edges per (window, table) run, so a snake
    # deal by descending in-degree makes each window's edge count nearly
    # equal across cores (the output [G, C] is permutation-invariant).
    indeg = np.bincount(cols, minlength=N)
    order = np.argsort(-indeg, kind="stable")
    rounds, lanes = np.divmod(np.arange(N), d.ncores)
    lanes = np.where(rounds % 2 == 0, lanes, d.ncores - 1 - lanes)
    asg_core = np.empty(N, np.int64)
    asg_slot = np.empty(N, np.int64)
    asg_core[order] = lanes
    # stride-scatter the degree-ranked rounds over slots (97 coprime to
    # shard) so each 64-slot window mixes ranks: window edge counts stay
    # near-equal both across cores and across windows
    asg_slot[order] = (rounds * 97) % d.shard
    inv = np.empty((d.ncores, d.shard), np.int64)
    inv[asg_core, asg_slot] = np.arange(N)
    pl.inv = inv

    core_of = asg_core[cols]
    # remap source node id to its position in table A or B of the split
    # all-gathered y tables (half 0 = table A, half 1 = table B); the stored
    # index is already table-local.
    k_src = asg_core[rows]
    r_loc = asg_slot[rows]
    lo_all = r_loc < d.S0
    pid = np.where(lo_all, k_src * d.S0 + r_loc,
                   k_src * d.S1 + (r_loc - d.S0))
    # Per (core, window, half) edge lists, edges sorted by local dest.
    dst_slot = asg_slot[cols]
    per_core = []
    for k in range(d.ncores):
        m = core_of == k
        r = pid[m]
        c = dst_slot[m]
        lo = lo_all[m]
        order = np.argsort(c, kind="stable")
        r, c, lo = r[order], c[order], lo[order]
        w = c // WIN
        lists = {}
        # bucket by (window, half) preserving dest order
        for half_id, mask in ((0, lo), (1, ~lo)):
            rw, cw, ww = r[mask], c[mask], w[mask]
            # indices where window changes
            for wi in range(d.nwin):
                sel = ww == wi
                lists[(wi, half_id)] = (rw[sel], cw[sel])
        per_core.append(lists)

    # Shared chunk schedule: R[w][half] = max over cores of ceil(count/128)
    R = np.zeros((d.nwin, 2), dtype=np.int64)
    for k in range(d.ncores):
        for (wi, hf), (rw, cw) in per_core[k].items():
            R[wi, hf] = max(R[wi, hf], ceil_div(len(rw), P))
    R = np.maximum(R, 0)
    # every window must be initialized in PSUM: ensure at least one chunk
    for wi in range(d.nwin):
        if R[wi].sum() == 0:
            R[wi, 0] = 1

    # Build the chunk stream: per block: [lo chunks (w asc)] ++ [hi chunks]
    stream = []          # list of (window, half) per chunk position
    groups = []          # (block, half, chunk_start, chunk_count)
    for b in range(d.nblk):
        wlo = b * BLOCK_WINS
        whi = min(wlo + BLOCK_WINS, d.nwin)
        for hf in (0, 1):
            g0 = len(stream)
            for wi in range(wlo, whi):
                for _ in range(R[wi, hf]):
                    stream.append((wi, hf))
            groups.append((b, hf, g0, len(stream) - g0))
    C_grid = len(stream)
    tot_slots = C_grid * P

    # start/stop flags: matmul start=True zeroes the ENTIRE 2KB PSUM strip of
    # its output partitions, so exactly one start per (block, parity strip) --
    # the first chunk in stream order touching that strip; stop on the last.
    # all-accumulate scheme: the block PSUM tile is DVE-memset to zero, every
    # matmul uses start=False (accumulate). A start=True would zero the whole
    # 2KB PSUM strip of its partitions, wiping sibling windows in the bank.
    start_flag = np.zeros(C_grid, dtype=bool)
    stop_flag = np.zeros(C_grid, dtype=bool)

    # Fill per-core slot data. Pad slots keep idx=-1: the gather ucode skips
    # trailing negative indices, so per-(window,half)-run gather instructions
    # transfer only each core's real edges (padding varies per core).
    idx_all = np.full((d.ncores, tot_slots), -1, dtype=np.int16)
    A_all = np.full((d.ncores, C_grid, P), 300.0, dtype=np.float32)
    # chunk positions per (window, half) in stream order:
    pos_of = {}
    for pos, key in enumerate(stream):
        pos_of.setdefault(key, []).append(pos)
    # shared per-run valid count V = max over cores of real edges in the run.
    # num_idxs_reg must equal the count of non-negative indices and is a
    # shared immediate, so every core pads its run with dummy (idx=0, no
    # dest) slots up to V; slots beyond V keep idx=-1 and are skipped.
    V = np.zeros((d.nwin, 2), dtype=np.int64)
    for k in range(d.ncores):
        for (wi, hf), (rw, cw) in per_core[k].items():
            V[wi, hf] = max(V[wi, hf], len(rw))
    for wi in range(d.nwin):
        if V[wi].sum() == 0:
            V[wi, 0] = 1      # matches the R fixup: run exists, 1 dummy slot
    for k in range(d.ncores):
        for (wi, hf), (rw, cw) in per_core[k].items():
            n = len(rw)
            positions = pos_of.get((wi, hf), [])
            if not positions:
                assert n == 0
                continue
            assert n <= len(positions) * P
            vals = rw          # already table-local (split tables A/B)
            crel = cw - wi * WIN
            for j, pos in enumerate(positions):
                a, bnd = j * P, min((j + 1) * P, n)
                if a < n:
                    cnt = bnd - a
                    idx_all[k, pos * P: pos * P + cnt] = \
                        vals[a:bnd].astype(np.int16)
                    A_all[k, pos, :cnt] = crel[a:bnd].astype(np.float32)
            # dummy-valid padding up to the shared count V
            run0 = positions[0] * P
            idx_all[k, run0 + n: run0 + V[wi, hf]] = 0

    # wrap idx to the [128, tot_slots//16] layout dma_gather wants:
    # slot i -> [16*c + i%16, i//16] for every q7 core c
    S16 = tot_slots // 16
    idx_wrapped = np.zeros((d.ncores, P, S16), dtype=np.int16)
    for k in range(d.ncores):
        w16 = idx_all[k].reshape(S16, 16).T  # [16, S16]
        idx_wrapped[k] = np.tile(w16, (8, 1))

    # A matrix in [128 partitions=slot%128, C_grid] layout
    A_pt = np.transpose(A_all, (0, 2, 1)).astype(BF16)  # [cores, 128, C_grid]

    # per-core node-major helper arrays
    dinv_pt = np.zeros((d.ncores, P, d.ntile), dtype=np.float32)
    pool_pt = np.zeros((d.ncores, P, d.ntile, d.G), dtype=np.float32)
    for k in range(d.ncores):
        for t in range(d.ntile):
            for p in range(P):
                n0 = t * P + p
                if n0 < d.shard:
                    node = inv[k, n0]
                    dinv_pt[k, p, t] = dinv[node]
                    pool_pt[k, p, t, batch[node]] = 1.0

    cnts = np.bincount(batch, minlength=d.G).astype(np.float32)
    inv_cnt = (1.0 / np.maximum(cnts, 1.0)).reshape(d.G, 1)

    pl.R, pl.stream, pl.groups, pl.V = R, stream, groups, V
    pl.C_grid, pl.tot_slots = C_grid, tot_slots
    pl.start_flag, pl.stop_flag = start_flag, stop_flag
    pl.idx_wrapped, pl.A_pt = idx_wrapped, A_pt
    pl.dinv_pt, pl.pool_pt, pl.inv_cnt = dinv_pt, pool_pt.reshape(d.ncores, P, -1), inv_cnt
    pl.max_lo_chunks = max(g[3] for g in groups if g[1] == 0)
    pl.max_hi_chunks = max(g[3] for g in groups if g[1] == 1)
    blk_tot = {}
    for b, hf, g0, gc in groups:
        blk_tot[b] = blk_tot.get(b, 0) + gc
    pl.max_blk_chunks = max(blk_tot.values())
    return pl


# ----------------------------------------------------------------------------
# Bass program
# ----------------------------------------------------------------------------

def build_program(d: Dims, pl: Plan, debug=False, repeat=1, ablate=()):
    nc = bacc.Bacc("TRN2", target_bir_lowering=False, debug=False,
                   num_devices=d.ncores, num_swdge_queues=4)
    f32, bf16, i16 = mybir.dt.float32, mybir.dt.bfloat16, mybir.dt.int16

    def din(name, shape, dt=f32):
        return nc.dram_tensor(name, shape, dt, kind="ExternalInput").ap()

    xt = din("xt", [d.F, d.shard])
    W1 = din("W1", [d.F, d.H])
    W2 = din("W2", [d.H, d.H], bf16)
    Wc = din("Wc", [d.H, d.C])
    g1 = din("g1", [d.H, 1])
    be1 = din("be1", [d.H, 1])
    g2 = din("g2", [d.H, 1])
    be2 = din("be2", [d.H, 1])
    idx_d = din("idx", [P, pl.tot_slots // 16], i16)
    if "gathpair" in ablate or "gathhalf" in ablate:
        idxh_d = din("idxh", [P, pl.tot_slots // 32], i16)
    A_d = din("A", [P, pl.C_grid], bf16)
    dinv_d = din("dinv_pt", [P, d.ntile])
    pool_d = din("pool_pt", [P, d.ntile * d.G])
    invc_d = din("inv_cnt", [d.G, 1])
    bcr_d = din("bc_rep", [d.G, d.C])
    iota_d = din("iota", [P, WIN], bf16)
    ident_d = din("ident", [P, P])
    out_d = nc.dram_tensor("out", [d.G, d.C], f32, kind="ExternalOutput").ap()
    if debug:
        dbg_agg = nc.dram_tensor("dbg_agg", [P, d.ntile * d.H], f32,
                                 kind="ExternalOutput").ap()
        dbg_h = nc.dram_tensor("dbg_h", [d.H, d.ntile * P], f32,
                               kind="ExternalOutput").ap()
        dbg_y = nc.dram_tensor("dbg_y", [d.npadA + d.npadB, d.H], f32,
                               kind="ExternalOutput").ap()

    rg = [list(range(d.ncores))]

    with tile.TileContext(nc) as tc:
        with (
            tc.tile_pool(name="const", bufs=1) as cpool,
            tc.tile_pool(name="work", bufs=2) as wpool,
            tc.tile_pool(name="glo", bufs=4) as gpool_lo,
            tc.tile_pool(name="ghi", bufs=4) as gpool_hi,
            tc.tile_pool(name="spool", bufs=2) as spool,
            tc.tile_pool(name="big", bufs=1) as bigpool,
            tc.tile_pool(name="pseg", bufs=3, space="PSUM") as pseg,
            tc.tile_pool(name="pmm", bufs=2, space="PSUM") as pmm,
            tc.tile_pool(name="ptr", bufs=3, space="PSUM") as ptr,
            tc.tile_pool(name="dram", bufs=1, space="DRAM") as dpool,
            tc.tile_pool(name="dram_y", bufs=2, space="DRAM") as ypool,
            tc.tile_pool(name="dram_so", bufs=2, space="DRAM") as sopool,
        ):
            # ---- load constants ----
            def cload(ap, shape, dt=f32, name=None):
                t = cpool.tile(shape, dt, tag=name)
                nc.sync.dma_start(out=t[:], in_=ap)
                return t

            W1_s = cload(W1[:], [d.F, d.H], name="W1")
            W2_s = cload(W2[:], [d.H, d.H], bf16, name="W2")
            Wc_s = cload(Wc[:], [d.H, d.C], name="Wc")
            g1_s = cload(g1[:], [d.H, 1], name="g1")
            be1_s = cload(be1[:], [d.H, 1], name="be1")
            g2_s = cload(g2[:], [d.H, 1], name="g2")
            be2_s = cload(be2[:], [d.H, 1], name="be2")
            idx_s = cload(idx_d[:], [P, pl.tot_slots // 16], i16, name="idx")
            if "gathpair" in ablate or "gathhalf" in ablate:
                idxh_s = cload(idxh_d[:], [P, pl.tot_slots // 32], i16,
                               name="idxh")
            A_s = cload(A_d[:], [P, pl.C_grid], bf16, name="A")
            dinv_s = cload(dinv_d[:], [P, d.ntile], name="dinv")
            pool_s = cload(pool_d[:], [P, d.ntile * d.G], name="pool")
            invc_s = cload(invc_d[:], [d.G, 1], name="invc")
            bcr_s = cload(bcr_d[:], [d.G, d.C], name="bcr")
            iota_s = cload(iota_d[:], [P, WIN], bf16, name="iota")
            ident_s = cload(ident_d[:], [P, P], name="ident")

            # pool matrix as bf16 for matmul
            pool_bf = cpool.tile([P, d.ntile * d.G], bf16, tag="poolbf")
            nc.vector.tensor_copy(out=pool_bf[:], in_=pool_s[:])

            eps_s = cpool.tile([d.H, 1], f32, tag="eps")
            nc.vector.memset(eps_s[:], EPS)
            ident_bf = cpool.tile([P, P], bf16, tag="identbf")
            nc.vector.tensor_copy(out=ident_bf[:], in_=ident_s[:])

            # ---- internal DRAM for collectives ----
            adsp = "Local" if "nosharedout" in ablate else "Shared"
            y_own = dpool.tile([d.shard_pad, d.H], bf16)
            stats_in = dpool.tile([d.H, 2], f32)
            pool_in = dpool.tile([d.G, d.H], f32)
            pool_out = dpool.tile([d.G, d.H], f32, addr_space=adsp)

            h_fm = None  # feature-major relu'd activations [H, shard]
            gq = [0]
            qload = [0, 0, 0, 0]  # greedy per-queue descriptor balance

            # one-time zero of the gather pool buffers: slots skipped by the
            # ucode (trailing idx=-1) leave SBUF untouched, and S=0 only
            # protects against finite garbage (0*NaN would poison PSUM)
            for gp, mg, tg in ((gpool_lo, pl.max_lo_chunks, "g0"),
                               (gpool_hi, pl.max_hi_chunks, "g1")):
                for _ in range(4):  # must touch every pool buffer
                    zt = gp.tile([P, mg, d.H], bf16, tag=tg, name=f"z{tg}")
                    nc.vector.memset(zt[:], 0.0)

            for rep in range(repeat):
              for layer in range(2):
                y_fullA = ypool.tile([d.npadA, d.H], bf16, addr_space=adsp,
                                     name=f"y_fullA_r{rep}l{layer}")
                y_fullB = ypool.tile([d.npadB, d.H], bf16, addr_space=adsp,
                                     name=f"y_fullB_r{rep}l{layer}")
                stats_out = sopool.tile([d.H, 2], f32, addr_space=adsp,
                                        name=f"stats_out_r{rep}l{layer}")
                # ---------- y = dinv * (X @ W)  (own shard, node-major) ----
                # staged: compute rows [0,S0) then AllGather table A, then
                # rows [S0,shard) and AllGather table B, so the A-pass
                # gather/segmm below overlaps the B AllGather. The node-major
                # y stays resident in SBUF for the self-loop contribution.
                y_sb = bigpool.tile([P, d.ntile, d.H], bf16, tag="y_sb")
                n_mt = ceil_div(d.shard, 512)
                for phase in (0, 1):
                    mtr = (range(0, d.S0 // 512) if phase == 0
                           else range(d.S0 // 512, n_mt))
                    for mt in mtr:
                        c0 = mt * 512
                        cw = min(512, d.shard - c0)
                        nst = ceil_div(cw, P)
                        if layer == 0:
                            rhs_t = wpool.tile([d.F, 512], f32, tag="xt_t")
                            nc.sync.dma_start(out=rhs_t[:, :cw],
                                              in_=xt[:, c0:c0 + cw])
                            lhsT, rhs_ap = W1_s[:, :], rhs_t[:, :cw]
                        else:
                            lhsT, rhs_ap = W2_s[:, :], h_fm[:, c0:c0 + cw]
                        xw_ps = pmm.tile([d.H, 512], f32, tag="xw")
                        nc.tensor.matmul(out=xw_ps[:, :cw], lhsT=lhsT,
                                         rhs=rhs_ap, start=True, stop=True)
                        xw_sb = wpool.tile([d.H, 512], f32, tag="xw_sb")
                        nc.scalar.copy(out=xw_sb[:, :cw], in_=xw_ps[:, :cw])
                        # transpose 128-node subtiles; dinv scale in ACT evac
                        for st in range(nst):
                            t_global = mt * 4 + st
                            n0 = st * P
                            nw = min(P, cw - n0)
                            tr_ps = ptr.tile([P, d.H], f32, tag="ptr")
                            nc.tensor.transpose(out=tr_ps[:nw, :],
                                                in_=xw_sb[:, n0:n0 + nw],
                                                identity=ident_s[:])
                            nc.scalar.mul(out=y_sb[:nw, t_global, :],
                                          in_=tr_ps[:nw, :],
                                          mul=dinv_s[:nw,
                                                     t_global:t_global + 1])
                        nc.sync.dma_start(
                            out=y_own[c0:c0 + nst * P, :].rearrange(
                                "(t p) f -> p t f", p=P),
                            in_=y_sb[:, mt * 4:mt * 4 + nst, :])
                    y_in = (y_own[0:d.S0, :] if phase == 0
                            else y_own[d.S0:d.shard_pad, :])
                    y_out = y_fullA if phase == 0 else y_fullB
                    if "nogather_collective" in ablate:
                        nc.sync.dma_start(
                            out=y_out[0:(d.S0 if phase == 0 else d.S1), :],
                            in_=y_in)
                    else:
                        nc.gpsimd.collective_compute(
                            "AllGather", mybir.AluOpType.bypass,
                            replica_groups=rg,
                            ins=[y_in.opt()], outs=[y_out.opt()])

                # ---------- gather + segment matmul: pass A, then pass B ----
                probe = ("gathpair" in ablate) or ("gathhalf" in ablate)
                agg_dm = bigpool.tile([P, d.ntile, d.H], f32, tag="agg_dm")
                # feature-major bf16 copy of agg + BN stats, produced
                # incrementally during pass B (overlaps gather DMA)
                agg_fm = bigpool.tile([d.H, d.ntile * P], bf16, tag="agg_fm")
                s1p = wpool.tile([d.H, d.ntile], f32, tag="s1p")
                s2p = wpool.tile([d.H, d.ntile], f32, tag="s2p")
                scratch = wpool.tile([d.H, P], f32, tag="scr")
                if d.shard % P:
                    nc.vector.memset(agg_dm[:, d.ntile - 1, :], 0.0)
                for hf in (0, 1):
                    gpool = gpool_lo if hf == 0 else gpool_hi
                    ysrc = (y_fullA if hf == 0 else y_fullB)[:, :]
                    mgc = pl.max_lo_chunks if hf == 0 else pl.max_hi_chunks
                    for b in range(d.nblk):
                        wlo = b * BLOCK_WINS
                        whi = min(wlo + BLOCK_WINS, d.nwin)
                        _, _, g0, gcnt = pl.groups[2 * b + hf]
                        empty_b = hf == 1 and gcnt == 0
                        blk_ps = None
                        if not empty_b:
                            blk_ps = pseg.tile([P, 4 * d.H], f32, tag="seg")
                            nc.vector.memset(blk_ps[:], 0.0)
                        gt = None
                        if (gcnt and not empty_b
                                and "nodmagather" not in ablate and not probe):
                            gt = gpool.tile([P, mgc, d.H], bf16, tag=f"g{hf}")
                            if "contiggather" in ablate:
                                nc.sync.dma_start(
                                    out=gt[:, :gcnt, :],
                                    in_=ysrc[0:gcnt * P, :].rearrange(
                                        "(s p) f -> p s f", p=P))
                            else:
                                # one gather per window run: each core's
                                # padding is a trailing run of idx=-1 slots,
                                # which the ucode skips (no transfer)
                                roff = 0
                                for wi in range(wlo, whi):
                                    rc = int(pl.R[wi, hf])
                                    if rc == 0:
                                        continue
                                    V_run = (rc * P if PAD0
                                             else int(pl.V[wi, hf]))
                                    # split the run into small pieces across
                                    # queues; valid slots are a prefix of the
                                    # run, so each piece's reg count is exact
                                    # and empty pieces are skipped entirely
                                    pc = 0
                                    while pc < rc:
                                        pcw = min(PIECE_CHUNKS, rc - pc)
                                        reg = max(0, min(V_run - pc * P,
                                                         pcw * P))
                                        if reg > 0:
                                            ns_pp = pcw * P
                                            s0 = (g0 + roff + pc) * P
                                            qn = gq[0] % 4
                                            nc.gpsimd.dma_gather(
                                                out_ap=gt[:, roff + pc:
                                                          roff + pc + pcw, :],
                                                in_ap=ysrc,
                                                idxs_ap=idx_s[
                                                    :, s0 // 16:
                                                    (s0 + ns_pp) // 16],
                                                num_idxs=ns_pp,
                                                num_idxs_reg=reg,
                                                elem_size=d.H,
                                                single_packet=False,
                                                queue_num=qn,
                                            )
                                            gq[0] += 1
                                        pc += pcw
                                    roff += rc
                                assert roff == gcnt
                        elif gcnt and probe:
                            # timing probes: same bytes/half bytes with half
                            # the descriptors; gathered data unused (implies
                            # no segmm matmuls for this pass)
                            pair = "gathpair" in ablate
                            esz = 2 * d.H if pair else d.H
                            gtp = gpool.tile([P, ceil_div(mgc, 2), esz], bf16,
                                             tag=f"gp{hf}")
                            ns_p = gcnt * P
                            ns_h = ns_p // 2
                            ysrc_p = (ysrc.rearrange("(a two) f -> a (two f)",
                                                     two=2) if pair else ysrc)
                            nc.gpsimd.dma_gather(
                                out_ap=gtp[:, 0:ceil_div(ns_h, P), :],
                                in_ap=ysrc_p,
                                idxs_ap=idxh_s[:, (g0 * P) // 32:
                                               (g0 * P) // 32 + ns_h // 16],
                                num_idxs=ns_h,
                                num_idxs_reg=ns_h,
                                elem_size=esz,
                                single_packet=False,
                                queue_num=gq[0] % 4,
                            )
                            gq[0] += 1
                        if gcnt and gt is not None:
                            S_t = spool.tile(
                                [P, max(pl.max_lo_chunks, pl.max_hi_chunks),
                                 WIN], bf16, tag="S")
                            a_b = A_s[:, g0:g0 + gcnt].unsqueeze(2) \
                                .broadcast_to([P, gcnt, WIN])
                            i_b = iota_s[:].unsqueeze(1) \
                                .broadcast_to([P, gcnt, WIN])
                            nc.vector.tensor_tensor(out=S_t[:, :gcnt, :],
                                                    in0=a_b, in1=i_b,
                                                    op=mybir.AluOpType.is_equal)
                            for pos in (() if "nosegmm" in ablate
                                        else range(g0, g0 + gcnt)):
                                wi, _hx = pl.stream[pos]
                                lc = pos - g0
                                w_in_b = wi - wlo
                                wpp = P // WIN
                                pof = WIN * (w_in_b % wpp)
                                fof = d.H * (w_in_b // wpp)
                                nc.tensor.matmul(
                                    out=blk_ps[pof:pof + WIN, fof:fof + d.H],
                                    lhsT=S_t[:, lc, :],
                                    rhs=gt[:, lc, :],
                                    start=False, stop=False,
                                    skip_group_check=True,
                                )
                        # evacuate: dest-major agg with dinv scaling; pass B
                        # accumulates on top of pass A, then immediately
                        # transposes each finished tile for BN stats + the
                        # feature-major agg copy (overlaps later gathers)
                        for w4 in range(ceil_div((whi - wlo) * WIN, P)):
                            t_global = (BLOCK_WINS * WIN // P) * b + w4
                            nw = min(P, d.shard - t_global * P)
                            if hf == 0:
                                nc.scalar.mul(
                                    out=agg_dm[:nw, t_global, :],
                                    in_=blk_ps[:nw, w4 * d.H:(w4 + 1) * d.H],
                                    mul=dinv_s[:nw, t_global:t_global + 1])
                                # direct self-loop term: dinv^2 * xw = dinv * y
                                # (y already carries one dinv factor)
                                slt = wpool.tile([P, d.H], f32, tag="slt")
                                nc.scalar.mul(
                                    out=slt[:nw, :],
                                    in_=y_sb[:nw, t_global, :],
                                    mul=dinv_s[:nw, t_global:t_global + 1])
                                nc.vector.tensor_tensor(
                                    out=agg_dm[:nw, t_global, :],
                                    in0=agg_dm[:nw, t_global, :],
                                    in1=slt[:nw, :],
                                    op=mybir.AluOpType.add)
                                continue
                            if not empty_b:
                                evB = wpool.tile([P, d.H], f32, tag="evB")
                                nc.scalar.mul(
                                    out=evB[:nw, :],
                                    in_=blk_ps[:nw, w4 * d.H:(w4 + 1) * d.H],
                                    mul=dinv_s[:nw, t_global:t_global + 1])
                                nc.vector.tensor_tensor(
                                    out=agg_dm[:nw, t_global, :],
                                    in0=agg_dm[:nw, t_global, :],
                                    in1=evB[:nw, :],
                                    op=mybir.AluOpType.add)
                            t = t_global
                            tr_ps = ptr.tile([d.H, P], f32, tag="ptr")
                            nc.tensor.transpose(out=tr_ps[:, :],
                                                in_=agg_dm[:, t, :],
                                                identity=ident_s[:])
                            nc.scalar.activation(
                                out=agg_fm[:, t * P:(t + 1) * P],
                                in_=tr_ps[:],
                                func=mybir.ActivationFunctionType.Copy,
                                accum_out=s1p[:, t:t + 1])
                            nc.scalar.activation(
                                out=scratch[:], in_=tr_ps[:],
                                func=mybir.ActivationFunctionType.Square,
                                accum_out=s2p[:, t:t + 1])

                stats_sb = wpool.tile([d.H, 2], f32, tag="stats")
                nc.vector.tensor_reduce(out=stats_sb[:, 0:1], in_=s1p[:],
                                        axis=mybir.AxisListType.X,
                                        op=mybir.AluOpType.add)
                nc.vector.tensor_reduce(out=stats_sb[:, 1:2], in_=s2p[:],
                                        axis=mybir.AxisListType.X,
                                        op=mybir.AluOpType.add)
                nc.sync.dma_start(out=stats_in[:], in_=stats_sb[:])
                if "nostatsar" in ablate:
                    nc.sync.dma_start(out=stats_out[:], in_=stats_in[:])
                else:
                    nc.gpsimd.collective_compute(
                        "AllReduce", mybir.AluOpType.add, replica_groups=rg,
                        ins=[stats_in.opt()], outs=[stats_out.opt()])
                stats_g = wpool.tile([d.H, 2], f32, tag="statsg")
                nc.sync.dma_start(out=stats_g[:], in_=stats_out[:])
                # mean/var -> scale/bias
                mv = wpool.tile([d.H, 6], f32, tag="mv")
                inv_n = 1.0 / d.N
                nc.vector.tensor_scalar(out=mv[:, 0:1], in0=stats_g[:, 0:1],
                                        scalar1=inv_n, scalar2=None,
                                        op0=mybir.AluOpType.mult)  # mean
                nc.vector.tensor_scalar(out=mv[:, 1:2], in0=stats_g[:, 1:2],
                                        scalar1=inv_n, scalar2=None,
                                        op0=mybir.AluOpType.mult)  # E[x^2]
                nc.vector.tensor_tensor(out=mv[:, 2:3], in0=mv[:, 0:1],
                                        in1=mv[:, 0:1],
                                        op=mybir.AluOpType.mult)   # mean^2
                nc.vector.tensor_tensor(out=mv[:, 2:3], in0=mv[:, 1:2],
                                        in1=mv[:, 2:3],
                                        op=mybir.AluOpType.subtract)  # var
                nc.scalar.activation(out=mv[:, 3:4], in_=mv[:, 2:3],
                                     func=mybir.ActivationFunctionType.Sqrt,
                                     bias=eps_s[:])                # std
                nc.vector.reciprocal(out=mv[:, 4:5], in_=mv[:, 3:4])
                gg = g1_s if layer == 0 else g2_s
                bb = be1_s if layer == 0 else be2_s
                nc.vector.tensor_tensor(out=mv[:, 4:5], in0=mv[:, 4:5],
                                        in1=gg[:], op=mybir.AluOpType.mult)
                # bias = be - mean*scale
                nc.vector.tensor_tensor(out=mv[:, 5:6], in0=mv[:, 0:1],
                                        in1=mv[:, 4:5],
                                        op=mybir.AluOpType.mult)
                nc.vector.tensor_tensor(out=mv[:, 5:6], in0=bb[:],
                                        in1=mv[:, 5:6],
                                        op=mybir.AluOpType.subtract)
                if debug and layer == 0:
                    for tbl, base in ((y_fullA, 0), (y_fullB, d.npadA)):
                        for t in range(tbl.shape[0] // P):
                            dbg_y_bf = wpool.tile([P, d.H], bf16,
                                                  tag="dbgybf")
                            dbg_y_sb = wpool.tile([P, d.H], f32, tag="dbgy")
                            nc.sync.dma_start(
                                out=dbg_y_bf[:],
                                in_=tbl[t * P:(t + 1) * P, :])
                            nc.vector.tensor_copy(out=dbg_y_sb[:],
                                                  in_=dbg_y_bf[:])
                            nc.sync.dma_start(
                                out=dbg_y[base + t * P:base + (t + 1) * P, :],
                                in_=dbg_y_sb[:])
                    nc.sync.dma_start(
                        out=dbg_agg[:],
                        in_=agg_dm[:].rearrange("p t f -> p (t f)"))
                h_fm = bigpool.tile([d.H, d.ntile * P], bf16, tag="h_fm")
                for t in range(d.ntile):
                    nc.scalar.activation(out=h_fm[:, t * P:(t + 1) * P],
                                         in_=agg_fm[:, t * P:(t + 1) * P],
                                         func=mybir.ActivationFunctionType.Relu,
                                         scale=mv[:, 4:5], bias=mv[:, 5:6])

            if debug:
                dbg_h_sb = wpool.tile([d.H, d.ntile * P], f32, tag="dbgh")
                nc.vector.tensor_copy(out=dbg_h_sb[:], in_=h_fm[:])
                nc.sync.dma_start(out=dbg_h[:], in_=dbg_h_sb[:])
            # ---------- pooling ----------
            # node-major h2 tiles via transpose, then matmul with pool matrix
            pool_ps = ptr.tile([d.G, d.H], f32, tag="ptr")
            for t in range(d.ntile):
                tr_ps = ptr.tile([P, d.H], bf16, tag="ptr")
                nc.tensor.transpose(out=tr_ps[:, :],
                                    in_=h_fm[:, t * P:(t + 1) * P],
                                    identity=ident_bf[:])
                h_dm = wpool.tile([P, d.H], bf16, tag="h_dm")
                nc.scalar.copy(out=h_dm[:], in_=tr_ps[:])
                nc.tensor.matmul(
                    out=pool_ps[:, :],
                    lhsT=pool_bf[:, t * d.G:(t + 1) * d.G],
                    rhs=h_dm[:],
                    start=(t == 0), stop=(t == d.ntile - 1))
            pool_sb = wpool.tile([d.G, d.H], f32, tag="poolsb")
            nc.vector.tensor_scalar(out=pool_sb[:], in0=pool_ps[:],
                                    scalar1=invc_s[:], scalar2=None,
                                    op0=mybir.AluOpType.mult)
            nc.sync.dma_start(out=pool_in[:], in_=pool_sb[:])
            if "nopoolar" in ablate:
                nc.sync.dma_start(out=pool_out[:], in_=pool_in[:])
            else:
                nc.gpsimd.collective_compute(
                    "AllReduce", mybir.AluOpType.add, replica_groups=rg,
                    ins=[pool_in.opt()], outs=[pool_out.opt()])
            pooled = wpool.tile([d.G, d.H], f32, tag="pooled")
            nc.sync.dma_start(out=pooled[:], in_=pool_out[:])
            # transpose pooled -> [H, G]
            pooled_t_ps = ptr.tile([d.H, d.G], f32, tag="ptr")
            nc.tensor.transpose(out=pooled_t_ps[:, :], in_=pooled[:],
                                identity=ident_s[:d.G, :d.G])
            pooled_t = wpool.tile([d.H, d.G], f32, tag="pooledtsb")
            nc.scalar.copy(out=pooled_t[:], in_=pooled_t_ps[:])
            out_ps = ptr.tile([d.G, d.C], f32, tag="ptr")
            nc.tensor.matmul(out=out_ps[:], lhsT=pooled_t[:], rhs=Wc_s[:],
                             start=True, stop=True)
            out_sb = wpool.tile([d.G, d.C], f32, tag="outsb")
            nc.vector.tensor_tensor(out=out_sb[:], in0=out_ps[:],
                                    in1=bcr_s[:], op=mybir.AluOpType.add)
            nc.scalar.activation(out=out_sb[:], in_=out_sb[:],
                                 func=mybir.ActivationFunctionType.Sigmoid)
            nc.sync.dma_start(out=out_d[:], in_=out_sb[:])

    nc.compile()
    return nc


# ----------------------------------------------------------------------------
# Entry point
# ----------------------------------------------------------------------------

def make_in_maps(d: Dims, pl: Plan, inputs):
    x = np.asarray(inputs["x"], np.float32)
    W1 = np.asarray(inputs["W1"], np.float32)
    W2 = np.asarray(inputs["W2"], np.float32)
    Wc = np.asarray(inputs["Wc"], np.float32)
    g1 = np.asarray(inputs["g1"], np.float32).reshape(d.H, 1)
    be1 = np.asarray(inputs["be1"], np.float32).reshape(d.H, 1)
    g2 = np.asarray(inputs["g2"], np.float32).reshape(d.H, 1)
    be2 = np.asarray(inputs["be2"], np.float32).reshape(d.H, 1)
    bc = np.asarray(inputs["bc"], np.float32)
    xt = np.ascontiguousarray(x.T)
    iota = np.tile(np.arange(WIN, dtype=np.float32), (P, 1)).astype(BF16)
    ident = np.eye(P, dtype=np.float32)
    bc_rep = np.tile(bc.reshape(1, d.C), (d.G, 1)).astype(np.float32)
    in_maps = []
    for k in range(d.ncores):
        in_maps.append({
            "xt": np.ascontiguousarray(xt[:, pl.inv[k]]),
            "W1": W1, "W2": W2.astype(BF16), "Wc": Wc,
            "g1": g1, "be1": be1, "g2": g2, "be2": be2,
            "idx": (np.maximum(pl.idx_wrapped[k], 0) if PAD0
                    else pl.idx_wrapped[k]),
            "idxh": np.maximum(pl.idx_wrapped[k][:, :pl.tot_slots // 32], 0)
                    // 2,
            "A": np.ascontiguousarray(pl.A_pt[k]),
            "dinv_pt": pl.dinv_pt[k],
            "pool_pt": pl.pool_pt[k],
            "inv_cnt": pl.inv_cnt,
            "bc_rep": bc_rep,
            "iota": iota,
            "ident": ident,
        })
    return in_maps


PAD0 = False  # True: pad slots gather row 0 instead of being skipped (-1)


def kernel(**inputs) -> np.ndarray:
    d = Dims()
    edge_index = np.asarray(inputs["edge_index"], np.int64)
    batch = np.asarray(inputs["batch"], np.int64)
    pl = make_plan(d, edge_index, batch)
    nc = build_program(d, pl)
    in_maps = make_in_maps(d, pl, inputs)
    res = run_bass_kernel_spmd(nc, in_maps, core_ids=list(range(d.ncores)))
    return np.asarray(res.results[0]["out"], np.float32)



# revision 56
# speedup vs baseline: 1.1801x; 1.0692x over previous
"""GCN (2-layer + BN + global mean pool + sigmoid readout) on 8 TRN2 NeuronCores.

Strategy (see spec sharding_hint): destinations (nodes) sharded across the 8
cores; each core aggregates messages for its node shard.  Per layer:

  y = dinv * (X @ W)            (node-major, bf16, exchanged via AllGather)
  agg[c] = dinv[c] * (sum_{e: col_e==c} y[row_e]  +  y[c])   (self loop direct)
  h = relu(BN(agg))

Perf structure (HW-measured: the SWDGE indexed gather is ~95% of runtime,
~4.7ns/descriptor across 4 queues; everything else hides under it):
 - y is exchanged as TWO row-range tables via Shared-output AllGathers (the
   fast collective path), so the A-table gather/segment pass overlaps the
   B-table AllGather.
 - One dma_gather piece per few chunks of each (window, table) run; each
   core's padding is a trailing run of idx=-1 slots the ucode skips, with a
   shared num_idxs_reg equal to the cross-core max real count (dummy idx=0
   slots make the count identical on every core).
 - Self loops never gather: their y stays SBUF-resident node-major and is
   added with one ACT mul + DVE add per tile.
 - The segment-sum is a TensorE matmul of each gathered 128-edge chunk
   against a one-hot S matrix built on DVE (is_equal vs iota), accumulated in
   PSUM per 512-destination block; BN stats transposes run inside pass B so
   they overlap the gather DMA.
The instruction stream is identical on all 8 cores (SPMD); all per-core
variation lives in the input data (indices, selection metadata, padding).
"""

import numpy as np
import ml_dtypes

import concourse.bacc as bacc
import concourse.bass as bass
import concourse.tile as tile
from concourse import mybir
from concourse.bass_utils import run_bass_kernel_spmd

BF16 = ml_dtypes.bfloat16
P = 128          # partitions / chunk size
WIN = 64         # dest window width (S matrix width)
BLOCK_WINS = 8   # windows per PSUM block (8*64 = 512 dests)
PIECE_CHUNKS = 99  # chunks per gather piece; whole runs (~640 desc) pipeline best
EPS = 1e-5


class Dims:
    def __init__(self, N=50000, E=800000, F=96, H=128, G=64, C=50,
                 ncores=8):
        assert N % ncores == 0
        self.N, self.E, self.F, self.H = N, E, F, H
        self.G, self.C = G, C
        self.ncores = ncores
        self.shard = N // ncores
        self.ntile = ceil_div(self.shard, P)  # node tiles per shard
        self.shard_pad = self.ntile * P       # padded shard rows in y table
        # local row-range split: table A = rows [0, S0), table B = [S0, pad).
        # Both 512-aligned (mt-group granularity); each table's all-gathered
        # row count stays within int16 index range for dma_gather.
        self.S0 = 3072
        self.S1 = self.shard_pad - self.S0
        self.npadA = self.S0 * ncores         # 24576 < 32768
        self.npadB = self.S1 * ncores         # 25600 < 32768
        assert self.npadA < 32768 and self.npadB < 32768
        self.nwin = ceil_div(self.shard, WIN)
        self.nblk = ceil_div(self.nwin, BLOCK_WINS)


def ceil_div(a, b):
    return (a + b - 1) // b


# ----------------------------------------------------------------------------
# Host planning: pure index/graph preprocessing (functions of edge_index/batch)
# ----------------------------------------------------------------------------

class Plan:
    pass


def make_plan(d: Dims, edge_index: np.ndarray, batch: np.ndarray) -> Plan:
    pl = Plan()
    N, E = d.N, d.E
    # self-loops are NOT routed through the gather: each core adds
    # dinv^2 * y for its own nodes directly on-chip. deg still counts them.
    rows = edge_index[0].astype(np.int64)
    cols = edge_index[1].astype(np.int64)
    deg = (np.bincount(cols, minlength=N) + 1).astype(np.float64)
    dinv = (1.0 / np.sqrt(np.maximum(deg, 1.0))).astype(np.float32)

    # Degree-balanced node -> (core, slot) assignment: the shared gather
    # schedule pays max-over-cores edges per (window, table) run, so a snake
    # deal by descending in-degree makes each window's edge count nearly
    # equal across cores (the output [G, C] is permutation-invariant).
    indeg = np.bincount(cols, minlength=N)
    order = np.argsort(-indeg, kind="stable")
    rounds, lanes = np.divmod(np.arange(N), d.ncores)
    lanes = np.where(rounds % 2 == 0, lanes, d.ncores - 1 - lanes)
    asg_core = np.empty(N, np.int64)
    asg_slot = np.empty(N, np.int64)
    asg_core[order] = lanes
    # stride-scatter the degree-ranked rounds over slots (97 coprime to
    # shard) so each 64-slot window mixes ranks: window edge counts stay
    # near-equal both across cores and across windows
    asg_slot[order] = (rounds * 97) % d.shard
    inv = np.empty((d.ncores, d.shard), np.int64)
    inv[asg_core, asg_slot] = np.arange(N)
    pl.inv = inv

    core_of = asg_core[cols]
    # remap source node id to its position in table A or B of the split
    # all-gathered y tables (half 0 = table A, half 1 = table B); the stored
    # index is already table-local.
    k_src = asg_core[rows]
    r_loc = asg_slot[rows]
    lo_all = r_loc < d.S0
    pid = np.where(lo_all, k_src * d.S0 + r_loc,
                   k_src * d.S1 + (r_loc - d.S0))
    # Per (core, window, half) edge lists, edges sorted by local dest.
    dst_slot = asg_slot[cols]
    per_core = []
    for k in range(d.ncores):
        m = core_of == k
        r = pid[m]
        c = dst_slot[m]
        lo = lo_all[m]
        order = np.argsort(c, kind="stable")
        r, c, lo = r[order], c[order], lo[order]
        w = c // WIN
        lists = {}
        # bucket by (window, half) preserving dest order
        for half_id, mask in ((0, lo), (1, ~lo)):
            rw, cw, ww = r[mask], c[mask], w[mask]
            # indices where window changes
            for wi in range(d.nwin):
                sel = ww == wi
                lists[(wi, half_id)] = (rw[sel], cw[sel])
        per_core.append(lists)

    # Shared chunk schedule: R[w][half] = max over cores of ceil(count/128)
    R = np.zeros((d.nwin, 2), dtype=np.int64)
    for k in range(d.ncores):
        for (wi, hf), (rw, cw) in per_core[k].items():
            R[wi, hf] = max(R[wi, hf], ceil_div(len(rw), P))
    R = np.maximum(R, 0)
    # every window must be initialized in PSUM: ensure at least one chunk
    for wi in range(d.nwin):
        if R[wi].sum() == 0:
            R[wi, 0] = 1

    # Build the chunk stream: per block: [lo chunks (w asc)] ++ [hi chunks]
    stream = []          # list of (window, half) per chunk position
    groups = []          # (block, half, chunk_start, chunk_count)
    for b in range(d.nblk):
        wlo = b * BLOCK_WINS
        whi = min(wlo + BLOCK_WINS, d.nwin)
        for hf in (0, 1):
            g0 = len(stream)
            for wi in range(wlo, whi):
                for _ in range(R[wi, hf]):
                    stream.append((wi, hf))
            groups.append((b, hf, g0, len(stream) - g0))
    C_grid = len(stream)
    tot_slots = C_grid * P

    # start/stop flags: matmul start=True zeroes the ENTIRE 2KB PSUM strip of
    # its output partitions, so exactly one start per (block, parity strip) --
    # the first chunk in stream order touching that strip; stop on the last.
    # all-accumulate scheme: the block PSUM tile is DVE-memset to zero, every
    # matmul uses start=False (accumulate). A start=True would zero the whole
    # 2KB PSUM strip of its partitions, wiping sibling windows in the bank.
    start_flag = np.zeros(C_grid, dtype=bool)
    stop_flag = np.zeros(C_grid, dtype=bool)

    # Fill per-core slot data. Pad slots keep idx=-1: the gather ucode skips
    # trailing negative indices, so per-(window,half)-run gather instructions
    # transfer only each core's real edges (padding varies per core).
    idx_all = np.full((d.ncores, tot_slots), -1, dtype=np.int16)
    A_all = np.full((d.ncores, C_grid, P), 300.0, dtype=np.float32)
    # chunk positions per (window, half) in stream order:
    pos_of = {}
    for pos, key in enumerate(stream):
        pos_of.setdefault(key, []).append(pos)
    # shared per-run valid count V = max over cores of real edges in the run.
    # num_idxs_reg must equal the count of non-negative indices and is a
    # shared immediate, so every core pads its run with dummy (idx=0, no
    # dest) slots up to V; slots beyond V keep idx=-1 and are skipped.
    V = np.zeros((d.nwin, 2), dtype=np.int64)
    for k in range(d.ncores):
        for (wi, hf), (rw, cw) in per_core[k].items():
            V[wi, hf] = max(V[wi, hf], len(rw))
    for wi in range(d.nwin):
        if V[wi].sum() == 0:
            V[wi, 0] = 1      # matches the R fixup: run exists, 1 dummy slot
    for k in range(d.ncores):
        for (wi, hf), (rw, cw) in per_core[k].items():
            n = len(rw)
            positions = pos_of.get((wi, hf), [])
            if not positions:
                assert n == 0
                continue
            assert n <= len(positions) * P
            vals = rw          # already table-local (split tables A/B)
            crel = cw - wi * WIN
            for j, pos in enumerate(positions):
                a, bnd = j * P, min((j + 1) * P, n)
                if a < n:
                    cnt = bnd - a
                    idx_all[k, pos * P: pos * P + cnt] = \
                        vals[a:bnd].astype(np.int16)
                    A_all[k, pos, :cnt] = crel[a:bnd].astype(np.float32)
            # dummy-valid padding up to the shared count V
            run0 = positions[0] * P
            idx_all[k, run0 + n: run0 + V[wi, hf]] = 0

    # wrap idx to the [128, tot_slots//16] layout dma_gather wants:
    # slot i -> [16*c + i%16, i//16] for every q7 core c
    S16 = tot_slots // 16
    idx_wrapped = np.zeros((d.ncores, P, S16), dtype=np.int16)
    for k in range(d.ncores):
        w16 = idx_all[k].reshape(S16, 16).T  # [16, S16]
        idx_wrapped[k] = np.tile(w16, (8, 1))

    # A matrix in [128 partitions=slot%128, C_grid] layout
    A_pt = np.transpose(A_all, (0, 2, 1)).astype(BF16)  # [cores, 128, C_grid]

    # per-core node-major helper arrays
    dinv_pt = np.zeros((d.ncores, P, d.ntile), dtype=np.float32)
    pool_pt = np.zeros((d.ncores, P, d.ntile, d.G), dtype=np.float32)
    for k in range(d.ncores):
        for t in range(d.ntile):
            for p in range(P):
                n0 = t * P + p
                if n0 < d.shard:
                    node = inv[k, n0]
                    dinv_pt[k, p, t] = dinv[node]
                    pool_pt[k, p, t, batch[node]] = 1.0

    cnts = np.bincount(batch, minlength=d.G).astype(np.float32)
    inv_cnt = (1.0 / np.maximum(cnts, 1.0)).reshape(d.G, 1)

    pl.R, pl.stream, pl.groups, pl.V = R, stream, groups, V
    pl.C_grid, pl.tot_slots = C_grid, tot_slots
    pl.start_flag, pl.stop_flag = start_flag, stop_flag
    pl.idx_wrapped, pl.A_pt = idx_wrapped, A_pt
    pl.dinv_pt, pl.pool_pt, pl.inv_cnt = dinv_pt, pool_pt.reshape(d.ncores, P, -1), inv_cnt
    pl.max_lo_chunks = max(g[3] for g in groups if g[1] == 0)
    pl.max_hi_chunks = max(g[3] for g in groups if g[1] == 1)
    blk_tot = {}
    for b, hf, g0, gc in groups:
        blk_tot[b] = blk_tot.get(b, 0) + gc
    pl.max_blk_chunks = max(blk_tot.values())
    return pl


# ----------------------------------------------------------------------------
# Bass program
# ----------------------------------------------------------------------------

def build_program(d: Dims, pl: Plan, debug=False, repeat=1, ablate=()):
    nc = bacc.Bacc("TRN2", target_bir_lowering=False, debug=False,
                   num_devices=d.ncores, num_swdge_queues=4)
    f32, bf16, i16 = mybir.dt.float32, mybir.dt.bfloat16, mybir.dt.int16

    def din(name, shape, dt=f32):
        return nc.dram_tensor(name, shape, dt, kind="ExternalInput").ap()

    xt = din("xt", [d.F, d.shard])
    W1 = din("W1", [d.F, d.H])
    W2 = din("W2", [d.H, d.H], bf16)
    Wc = din("Wc", [d.H, d.C])
    g1 = din("g1", [d.H, 1])
    be1 = din("be1", [d.H, 1])
    g2 = din("g2", [d.H, 1])
    be2 = din("be2", [d.H, 1])
    idx_d = din("idx", [P, pl.tot_slots // 16], i16)
    if "gathpair" in ablate or "gathhalf" in ablate:
        idxh_d = din("idxh", [P, pl.tot_slots // 32], i16)
    A_d = din("A", [P, pl.C_grid], bf16)
    dinv_d = din("dinv_pt", [P, d.ntile])
    pool_d = din("pool_pt", [P, d.ntile * d.G])
    invc_d = din("inv_cnt", [d.G, 1])
    bcr_d = din("bc_rep", [d.G, d.C])
    iota_d = din("iota", [P, WIN], bf16)
    ident_d = din("ident", [P, P])
    out_d = nc.dram_tensor("out", [d.G, d.C], f32, kind="ExternalOutput").ap()
    if debug:
        dbg_agg = nc.dram_tensor("dbg_agg", [P, d.ntile * d.H], f32,
                                 kind="ExternalOutput").ap()
        dbg_h = nc.dram_tensor("dbg_h", [d.H, d.ntile * P], f32,
                               kind="ExternalOutput").ap()
        dbg_y = nc.dram_tensor("dbg_y", [d.npadA + d.npadB, d.H], f32,
                               kind="ExternalOutput").ap()

    rg = [list(range(d.ncores))]

    with tile.TileContext(nc) as tc:
        with (
            tc.tile_pool(name="const", bufs=1) as cpool,
            tc.tile_pool(name="work", bufs=2) as wpool,
            tc.tile_pool(name="glo", bufs=4) as gpool_lo,
            tc.tile_pool(name="ghi", bufs=4) as gpool_hi,
            tc.tile_pool(name="spool", bufs=2) as spool,
            tc.tile_pool(name="big", bufs=1) as bigpool,
            tc.tile_pool(name="pseg", bufs=3, space="PSUM") as pseg,
            tc.tile_pool(name="pmm", bufs=2, space="PSUM") as pmm,
            tc.tile_pool(name="ptr", bufs=3, space="PSUM") as ptr,
            tc.tile_pool(name="dram", bufs=1, space="DRAM") as dpool,
            tc.tile_pool(name="dram_y", bufs=2, space="DRAM") as ypool,
            tc.tile_pool(name="dram_so", bufs=2, space="DRAM") as sopool,
        ):
            # ---- load constants ----
            def cload(ap, shape, dt=f32, name=None):
                t = cpool.tile(shape, dt, tag=name)
                nc.sync.dma_start(out=t[:], in_=ap)
                return t

            W1_s = cload(W1[:], [d.F, d.H], name="W1")
            W2_s = cload(W2[:], [d.H, d.H], bf16, name="W2")
            Wc_s = cload(Wc[:], [d.H, d.C], name="Wc")
            g1_s = cload(g1[:], [d.H, 1], name="g1")
            be1_s = cload(be1[:], [d.H, 1], name="be1")
            g2_s = cload(g2[:], [d.H, 1], name="g2")
            be2_s = cload(be2[:], [d.H, 1], name="be2")
            idx_s = cload(idx_d[:], [P, pl.tot_slots // 16], i16, name="idx")
            if "gathpair" in ablate or "gathhalf" in ablate:
                idxh_s = cload(idxh_d[:], [P, pl.tot_slots // 32], i16,
                               name="idxh")
            A_s = cload(A_d[:], [P, pl.C_grid], bf16, name="A")
            dinv_s = cload(dinv_d[:], [P, d.ntile], name="dinv")
            pool_s = cload(pool_d[:], [P, d.ntile * d.G], name="pool")
            invc_s = cload(invc_d[:], [d.G, 1], name="invc")
            bcr_s = cload(bcr_d[:], [d.G, d.C], name="bcr")
            iota_s = cload(iota_d[:], [P, WIN], bf16, name="iota")
            ident_s = cload(ident_d[:], [P, P], name="ident")

            # pool matrix as bf16 for matmul
            pool_bf = cpool.tile([P, d.ntile * d.G], bf16, tag="poolbf")
            nc.vector.tensor_copy(out=pool_bf[:], in_=pool_s[:])

            eps_s = cpool.tile([d.H, 1], f32, tag="eps")
            nc.vector.memset(eps_s[:], EPS)
            ident_bf = cpool.tile([P, P], bf16, tag="identbf")
            nc.vector.tensor_copy(out=ident_bf[:], in_=ident_s[:])

            # ---- internal DRAM for collectives ----
            adsp = "Local" if "nosharedout" in ablate else "Shared"
            y_own = dpool.tile([d.shard_pad, d.H], bf16)
            stats_in = dpool.tile([d.H, 2], f32)
            pool_in = dpool.tile([d.G, d.H], f32)
            pool_out = dpool.tile([d.G, d.H], f32, addr_space=adsp)

            h_fm = None  # feature-major relu'd activations [H, shard]
            gq = [0]
            qload = [0, 0, 0, 0]  # greedy per-queue descriptor balance

            # one-time zero of the gather pool buffers: slots skipped by the
            # ucode (trailing idx=-1) leave SBUF untouched, and S=0 only
            # protects against finite garbage (0*NaN would poison PSUM)
            for gp, mg, tg in ((gpool_lo, pl.max_lo_chunks, "g0"),
                               (gpool_hi, pl.max_hi_chunks, "g1")):
                for _ in range(4):  # must touch every pool buffer
                    zt = gp.tile([P, mg, d.H], bf16, tag=tg, name=f"z{tg}")
                    nc.vector.memset(zt[:], 0.0)

            for rep in range(repeat):
              for layer in range(2):
                y_fullA = ypool.tile([d.npadA, d.H], bf16, addr_space=adsp,
                                     name=f"y_fullA_r{rep}l{layer}")
                y_fullB = ypool.tile([d.npadB, d.H], bf16, addr_space=adsp,
                                     name=f"y_fullB_r{rep}l{layer}")
                stats_out = sopool.tile([d.H, 2], f32, addr_space=adsp,
                                        name=f"stats_out_r{rep}l{layer}")
                # ---------- y = dinv * (X @ W)  (own shard, node-major) ----
                # staged: compute rows [0,S0) then AllGather table A, then
                # rows [S0,shard) and AllGather table B, so the A-pass
                # gather/segmm below overlaps the B AllGather. The node-major
                # y stays resident in SBUF for the self-loop contribution.
                y_sb = bigpool.tile([P, d.ntile, d.H], bf16, tag="y_sb")
                n_mt = ceil_div(d.shard, 512)
                for phase in (0, 1):
                    mtr = (range(0, d.S0 // 512) if phase == 0
                           else range(d.S0 // 512, n_mt))
                    for mt in mtr:
                        c0 = mt * 512
                        cw = min(512, d.shard - c0)
                        nst = ceil_div(cw, P)
                        if layer == 0:
                            rhs_t = wpool.tile([d.F, 512], f32, tag="xt_t")
                            nc.sync.dma_start(out=rhs_t[:, :cw],
                                              in_=xt[:, c0:c0 + cw])
                            lhsT, rhs_ap = W1_s[:, :], rhs_t[:, :cw]
                        else:
                            lhsT, rhs_ap = W2_s[:, :], h_fm[:, c0:c0 + cw]
                        xw_ps = pmm.tile([d.H, 512], f32, tag="xw")
                        nc.tensor.matmul(out=xw_ps[:, :cw], lhsT=lhsT,
                                         rhs=rhs_ap, start=True, stop=True)
                        xw_sb = wpool.tile([d.H, 512], f32, tag="xw_sb")
                        nc.scalar.copy(out=xw_sb[:, :cw], in_=xw_ps[:, :cw])
                        # transpose 128-node subtiles; dinv scale in ACT evac
                        for st in range(nst):
                            t_global = mt * 4 + st
                            n0 = st * P
                            nw = min(P, cw - n0)
                            tr_ps = ptr.tile([P, d.H], f32, tag="ptr")
                            nc.tensor.transpose(out=tr_ps[:nw, :],
                                                in_=xw_sb[:, n0:n0 + nw],
                                                identity=ident_s[:])
                            nc.scalar.mul(out=y_sb[:nw, t_global, :],
                                          in_=tr_ps[:nw, :],
                                          mul=dinv_s[:nw,
                                                     t_global:t_global + 1])
                        nc.sync.dma_start(
                            out=y_own[c0:c0 + nst * P, :].rearrange(
                                "(t p) f -> p t f", p=P),
                            in_=y_sb[:, mt * 4:mt * 4 + nst, :])
                    y_in = (y_own[0:d.S0, :] if phase == 0
                            else y_own[d.S0:d.shard_pad, :])
                    y_out = y_fullA if phase == 0 else y_fullB
                    if "nogather_collective" in ablate:
                        nc.sync.dma_start(
                            out=y_out[0:(d.S0 if phase == 0 else d.S1), :],
                            in_=y_in)
                    else:
                        nc.gpsimd.collective_compute(
                            "AllGather", mybir.AluOpType.bypass,
                            replica_groups=rg,
                            ins=[y_in.opt()], outs=[y_out.opt()])

                # ---------- gather + segment matmul: pass A, then pass B ----
                probe = ("gathpair" in ablate) or ("gathhalf" in ablate)
                agg_dm = bigpool.tile([P, d.ntile, d.H], f32, tag="agg_dm")
                # feature-major bf16 copy of agg + BN stats, produced
                # incrementally during pass B (overlaps gather DMA)
                agg_fm = bigpool.tile([d.H, d.ntile * P], bf16, tag="agg_fm")
                s1p = wpool.tile([d.H, d.ntile], f32, tag="s1p")
                s2p = wpool.tile([d.H, d.ntile], f32, tag="s2p")
                scratch = wpool.tile([d.H, P], f32, tag="scr")
                if d.shard % P:
                    nc.vector.memset(agg_dm[:, d.ntile - 1, :], 0.0)
                for hf in (0, 1):
                    gpool = gpool_lo if hf == 0 else gpool_hi
                    ysrc = (y_fullA if hf == 0 else y_fullB)[:, :]
                    mgc = pl.max_lo_chunks if hf == 0 else pl.max_hi_chunks
                    for b in range(d.nblk):
                        wlo = b * BLOCK_WINS
                        whi = min(wlo + BLOCK_WINS, d.nwin)
                        _, _, g0, gcnt = pl.groups[2 * b + hf]
                        empty_b = hf == 1 and gcnt == 0
                        blk_ps = None
                        if not empty_b:
                            blk_ps = pseg.tile([P, 4 * d.H], f32, tag="seg")
                            nc.vector.memset(blk_ps[:], 0.0)
                        gt = None
                        if (gcnt and not empty_b
                                and "nodmagather" not in ablate and not probe):
                            gt = gpool.tile([P, mgc, d.H], bf16, tag=f"g{hf}")
                            if "contiggather" in ablate:
                                nc.sync.dma_start(
                                    out=gt[:, :gcnt, :],
                                    in_=ysrc[0:gcnt * P, :].rearrange(
                                        "(s p) f -> p s f", p=P))
                            else:
                                # one gather per window run: each core's
                                # padding is a trailing run of idx=-1 slots,
                                # which the ucode skips (no transfer)
                                roff = 0
                                for wi in range(wlo, whi):
                                    rc = int(pl.R[wi, hf])
                                    if rc == 0:
                                        continue
                                    V_run = (rc * P if PAD0
                                             else int(pl.V[wi, hf]))
                                    # split the run into small pieces across
                                    # queues; valid slots are a prefix of the
                                    # run, so each piece's reg count is exact
                                    # and empty pieces are skipped entirely
                                    pc = 0
                                    while pc < rc:
                                        pcw = min(PIECE_CHUNKS, rc - pc)
                                        reg = max(0, min(V_run - pc * P,
                                                         pcw * P))
                                        if reg > 0:
                                            ns_pp = pcw * P
                                            s0 = (g0 + roff + pc) * P
                                            qn = gq[0] % 4
                                            nc.gpsimd.dma_gather(
                                                out_ap=gt[:, roff + pc:
                                                          roff + pc + pcw, :],
                                                in_ap=ysrc,
                                                idxs_ap=idx_s[
                                                    :, s0 // 16:
                                                    (s0 + ns_pp) // 16],
                                                num_idxs=ns_pp,
                                                num_idxs_reg=reg,
                                                elem_size=d.H,
                                                single_packet=False,
                                                queue_num=qn,
                                            )
                                            gq[0] += 1
                                        pc += pcw
                                    roff += rc
                                assert roff == gcnt
                        elif gcnt and probe:
                            # timing probes: same bytes/half bytes with half
                            # the descriptors; gathered data unused (implies
                            # no segmm matmuls for this pass)
                            pair = "gathpair" in ablate
                            esz = 2 * d.H if pair else d.H
                            gtp = gpool.tile([P, ceil_div(mgc, 2), esz], bf16,
                                             tag=f"gp{hf}")
                            ns_p = gcnt * P
                            ns_h = ns_p // 2
                            ysrc_p = (ysrc.rearrange("(a two) f -> a (two f)",
                                                     two=2) if pair else ysrc)
                            nc.gpsimd.dma_gather(
                                out_ap=gtp[:, 0:ceil_div(ns_h, P), :],
                                in_ap=ysrc_p,
                                idxs_ap=idxh_s[:, (g0 * P) // 32:
                                               (g0 * P) // 32 + ns_h // 16],
                                num_idxs=ns_h,
                                num_idxs_reg=ns_h,
                                elem_size=esz,
                                single_packet=False,
                                queue_num=gq[0] % 4,
                            )
                            gq[0] += 1
                        if gcnt and gt is not None:
                            S_t = spool.tile(
                                [P, max(pl.max_lo_chunks, pl.max_hi_chunks),
                                 WIN], bf16, tag="S")
                            a_b = A_s[:, g0:g0 + gcnt].unsqueeze(2) \
                                .broadcast_to([P, gcnt, WIN])
                            i_b = iota_s[:].unsqueeze(1) \
                                .broadcast_to([P, gcnt, WIN])
                            nc.vector.tensor_tensor(out=S_t[:, :gcnt, :],
                                                    in0=a_b, in1=i_b,
                                                    op=mybir.AluOpType.is_equal)
                            for pos in (() if "nosegmm" in ablate
                                        else range(g0, g0 + gcnt)):
                                wi, _hx = pl.stream[pos]
                                lc = pos - g0
                                w_in_b = wi - wlo
                                wpp = P // WIN
                                pof = WIN * (w_in_b % wpp)
                                fof = d.H * (w_in_b // wpp)
                                nc.tensor.matmul(
                                    out=blk_ps[pof:pof + WIN, fof:fof + d.H],
                                    lhsT=S_t[:, lc, :],
                                    rhs=gt[:, lc, :],
                                    start=False, stop=False,
                                    skip_group_check=True,
                                )
                        # evacuate: dest-major agg with dinv scaling; pass B
                        # accumulates on top of pass A, then immediately
                        # transposes each finished tile for BN stats + the
                        # feature-major agg copy (overlaps later gathers)
                        for w4 in range(ceil_div((whi - wlo) * WIN, P)):
                            t_global = (BLOCK_WINS * WIN // P) * b + w4
                            nw = min(P, d.shard - t_global * P)
                            if hf == 0:
                                nc.scalar.mul(
                                    out=agg_dm[:nw, t_global, :],
                                    in_=blk_ps[:nw, w4 * d.H:(w4 + 1) * d.H],
                                    mul=dinv_s[:nw, t_global:t_global + 1])
                                # direct self-loop term: dinv^2 * xw = dinv * y
                                # (y already carries one dinv factor)
                                slt = wpool.tile([P, d.H], f32, tag="slt")
                                nc.scalar.mul(
                                    out=slt[:nw, :],
                                    in_=y_sb[:nw, t_global, :],
                                    mul=dinv_s[:nw, t_global:t_global + 1])
                                nc.vector.tensor_tensor(
                                    out=agg_dm[:nw, t_global, :],
                                    in0=agg_dm[:nw, t_global, :],
                                    in1=slt[:nw, :],
                                    op=mybir.AluOpType.add)
                                continue
                            if not empty_b:
                                evB = wpool.tile([P, d.H], f32, tag="evB")
                                nc.scalar.mul(
                                    out=evB[:nw, :],
                                    in_=blk_ps[:nw, w4 * d.H:(w4 + 1) * d.H],
                                    mul=dinv_s[:nw, t_global:t_global + 1])
                                nc.vector.tensor_tensor(
                                    out=agg_dm[:nw, t_global, :],
                                    in0=agg_dm[:nw, t_global, :],
                                    in1=evB[:nw, :],
                                    op=mybir.AluOpType.add)
                            t = t_global
                            tr_ps = ptr.tile([d.H, P], f32, tag="ptr")
                            nc.tensor.transpose(out=tr_ps[:, :],
                                                in_=agg_dm[:, t, :],
                                                identity=ident_s[:])
                            nc.scalar.activation(
                                out=agg_fm[:, t * P:(t + 1) * P],
                                in_=tr_ps[:],
                                func=mybir.ActivationFunctionType.Copy,
                                accum_out=s1p[:, t:t + 1])
                            nc.scalar.activation(
                                out=scratch[:], in_=tr_ps[:],
                                func=mybir.ActivationFunctionType.Square,
                                accum_out=s2p[:, t:t + 1])

                stats_sb = wpool.tile([d.H, 2], f32, tag="stats")
                nc.vector.tensor_reduce(out=stats_sb[:, 0:1], in_=s1p[:],
                                        axis=mybir.AxisListType.X,
                                        op=mybir.AluOpType.add)
                nc.vector.tensor_reduce(out=stats_sb[:, 1:2], in_=s2p[:],
                                        axis=mybir.AxisListType.X,
                                        op=mybir.AluOpType.add)
                nc.sync.dma_start(out=stats_in[:], in_=stats_sb[:])
                if "nostatsar" in ablate:
                    nc.sync.dma_start(out=stats_out[:], in_=stats_in[:])
                else:
                    nc.gpsimd.collective_compute(
                        "AllReduce", mybir.AluOpType.add, replica_groups=rg,
                        ins=[stats_in.opt()], outs=[stats_out.opt()])
                stats_g = wpool.tile([d.H, 2], f32, tag="statsg")
                nc.sync.dma_start(out=stats_g[:], in_=stats_out[:])
                # mean/var -> scale/bias
                mv = wpool.tile([d.H, 6], f32, tag="mv")
                inv_n = 1.0 / d.N
                nc.vector.tensor_scalar(out=mv[:, 0:1], in0=stats_g[:, 0:1],
                                        scalar1=inv_n, scalar2=None,
                                        op0=mybir.AluOpType.mult)  # mean
                nc.vector.tensor_scalar(out=mv[:, 1:2], in0=stats_g[:, 1:2],
                                        scalar1=inv_n, scalar2=None,
                                        op0=mybir.AluOpType.mult)  # E[x^2]
                nc.vector.tensor_tensor(out=mv[:, 2:3], in0=mv[:, 0:1],
                                        in1=mv[:, 0:1],
                                        op=mybir.AluOpType.mult)   # mean^2
                nc.vector.tensor_tensor(out=mv[:, 2:3], in0=mv[:, 1:2],
                                        in1=mv[:, 2:3],
                                        op=mybir.AluOpType.subtract)  # var
                nc.scalar.activation(out=mv[:, 3:4], in_=mv[:, 2:3],
                                     func=mybir.ActivationFunctionType.Sqrt,
                                     bias=eps_s[:])                # std
                nc.vector.reciprocal(out=mv[:, 4:5], in_=mv[:, 3:4])
                gg = g1_s if layer == 0 else g2_s
                bb = be1_s if layer == 0 else be2_s
                nc.vector.tensor_tensor(out=mv[:, 4:5], in0=mv[:, 4:5],
                                        in1=gg[:], op=mybir.AluOpType.mult)
                # bias = be - mean*scale
                nc.vector.tensor_tensor(out=mv[:, 5:6], in0=mv[:, 0:1],
                                        in1=mv[:, 4:5],
                                        op=mybir.AluOpType.mult)
                nc.vector.tensor_tensor(out=mv[:, 5:6], in0=bb[:],
                                        in1=mv[:, 5:6],
                                        op=mybir.AluOpType.subtract)
                if debug and layer == 0:
                    for tbl, base in ((y_fullA, 0), (y_fullB, d.npadA)):
                        for t in range(tbl.shape[0] // P):
                            dbg_y_bf = wpool.tile([P, d.H], bf16,
                                                  tag="dbgybf")
                            dbg_y_sb = wpool.tile([P, d.H], f32, tag="dbgy")
                            nc.sync.dma_start(
                                out=dbg_y_bf[:],
                                in_=tbl[t * P:(t + 1) * P, :])
                            nc.vector.tensor_copy(out=dbg_y_sb[:],
                                                  in_=dbg_y_bf[:])
                            nc.sync.dma_start(
                                out=dbg_y[base + t * P:base + (t + 1) * P, :],
                                in_=dbg_y_sb[:])
                    nc.sync.dma_start(
                        out=dbg_agg[:],
                        in_=agg_dm[:].rearrange("p t f -> p (t f)"))
                h_fm = bigpool.tile([d.H, d.ntile * P], bf16, tag="h_fm")
                for t in range(d.ntile):
                    nc.scalar.activation(out=h_fm[:, t * P:(t + 1) * P],
                                         in_=agg_fm[:, t * P:(t + 1) * P],
                                         func=mybir.ActivationFunctionType.Relu,
                                         scale=mv[:, 4:5], bias=mv[:, 5:6])

            if debug:
                dbg_h_sb = wpool.tile([d.H, d.ntile * P], f32, tag="dbgh")
                nc.vector.tensor_copy(out=dbg_h_sb[:], in_=h_fm[:])
                nc.sync.dma_start(out=dbg_h[:], in_=dbg_h_sb[:])
            # ---------- pooling ----------
            # node-major h2 tiles via transpose, then matmul with pool matrix
            pool_ps = ptr.tile([d.G, d.H], f32, tag="ptr")
            for t in range(d.ntile):
                tr_ps = ptr.tile([P, d.H], bf16, tag="ptr")
                nc.tensor.transpose(out=tr_ps[:, :],
                                    in_=h_fm[:, t * P:(t + 1) * P],
                                    identity=ident_bf[:])
                h_dm = wpool.tile([P, d.H], bf16, tag="h_dm")
                nc.scalar.copy(out=h_dm[:], in_=tr_ps[:])
                nc.tensor.matmul(
                    out=pool_ps[:, :],
                    lhsT=pool_bf[:, t * d.G:(t + 1) * d.G],
                    rhs=h_dm[:],
                    start=(t == 0), stop=(t == d.ntile - 1))
            pool_sb = wpool.tile([d.G, d.H], f32, tag="poolsb")
            nc.vector.tensor_scalar(out=pool_sb[:], in0=pool_ps[:],
                                    scalar1=invc_s[:], scalar2=None,
                                    op0=mybir.AluOpType.mult)
            nc.sync.dma_start(out=pool_in[:], in_=pool_sb[:])
            if "nopoolar" in ablate:
                nc.sync.dma_start(out=pool_out[:], in_=pool_in[:])
            else:
                nc.gpsimd.collective_compute(
                    "AllReduce", mybir.AluOpType.add, replica_groups=rg,
                    ins=[pool_in.opt()], outs=[pool_out.opt()])
            pooled = wpool.tile([d.G, d.H], f32, tag="pooled")
            nc.sync.dma_start(out=pooled[:], in_=pool_out[:])
            # transpose pooled -> [H, G]
            pooled_t_ps = ptr.tile([d.H, d.G], f32, tag="ptr")
            nc.tensor.transpose(out=pooled_t_ps[:, :], in_=pooled[:],
                                identity=ident_s[:d.G, :d.G])
            pooled_t = wpool.tile([d.H, d.G], f32, tag="pooledtsb")
            nc.scalar.copy(out=pooled_t[:], in_=pooled_t_ps[:])
            out_ps = ptr.tile([d.G, d.C], f32, tag="ptr")
            nc.tensor.matmul(out=out_ps[:], lhsT=pooled_t[:], rhs=Wc_s[:],
                             start=True, stop=True)
            out_sb = wpool.tile([d.G, d.C], f32, tag="outsb")
            nc.vector.tensor_tensor(out=out_sb[:], in0=out_ps[:],
                                    in1=bcr_s[:], op=mybir.AluOpType.add)
            nc.scalar.activation(out=out_sb[:], in_=out_sb[:],
                                 func=mybir.ActivationFunctionType.Sigmoid)
            nc.sync.dma_start(out=out_d[:], in_=out_sb[:])

    nc.compile()
    return nc


# ----------------------------------------------------------------------------
# Entry point
# ----------------------------------------------------------------------------

def make_in_maps(d: Dims, pl: Plan, inputs):
    x = np.asarray(inputs["x"], np.float32)
    W1 = np.asarray(inputs["W1"], np.float32)
    W2 = np.asarray(inputs["W2"], np.float32)
    Wc = np.asarray(inputs["Wc"], np.float32)
    g1 = np.asarray(inputs["g1"], np.float32).reshape(d.H, 1)
    be1 = np.asarray(inputs["be1"], np.float32).reshape(d.H, 1)
    g2 = np.asarray(inputs["g2"], np.float32).reshape(d.H, 1)
    be2 = np.asarray(inputs["be2"], np.float32).reshape(d.H, 1)
    bc = np.asarray(inputs["bc"], np.float32)
    xt = np.ascontiguousarray(x.T)
    iota = np.tile(np.arange(WIN, dtype=np.float32), (P, 1)).astype(BF16)
    ident = np.eye(P, dtype=np.float32)
    bc_rep = np.tile(bc.reshape(1, d.C), (d.G, 1)).astype(np.float32)
    in_maps = []
    for k in range(d.ncores):
        in_maps.append({
            "xt": np.ascontiguousarray(xt[:, pl.inv[k]]),
            "W1": W1, "W2": W2.astype(BF16), "Wc": Wc,
            "g1": g1, "be1": be1, "g2": g2, "be2": be2,
            "idx": (np.maximum(pl.idx_wrapped[k], 0) if PAD0
                    else pl.idx_wrapped[k]),
            "idxh": np.maximum(pl.idx_wrapped[k][:, :pl.tot_slots // 32], 0)
                    // 2,
            "A": np.ascontiguousarray(pl.A_pt[k]),
            "dinv_pt": pl.dinv_pt[k],
            "pool_pt": pl.pool_pt[k],
            "inv_cnt": pl.inv_cnt,
            "bc_rep": bc_rep,
            "iota": iota,
            "ident": ident,
        })
    return in_maps


PAD0 = False  # True: pad slots gather row 0 instead of being skipped (-1)


def kernel(**inputs) -> np.ndarray:
    d = Dims()
    edge_index = np.asarray(inputs["edge_index"], np.int64)
    batch = np.asarray(inputs["batch"], np.int64)
    pl = make_plan(d, edge_index, batch)
    nc = build_program(d, pl)
    in_maps = make_in_maps(d, pl, inputs)
    res = run_bass_kernel_spmd(nc, in_maps, core_ids=list(range(d.ncores)))
    return np.asarray(res.results[0]["out"], np.float32)



# revision 58
# speedup vs baseline: 2.2347x; 1.8936x over previous
"""GCN (2-layer + BN + global mean pool + sigmoid readout) on 8 TRN2 NeuronCores.

Strategy (see spec sharding_hint): destinations (nodes) sharded across the 8
cores; each core aggregates messages for its node shard.  Per layer:

  y = dinv * (X @ W)            (node-major, bf16, exchanged via AllGather)
  agg[c] = dinv[c] * (sum_{e: col_e==c} y[row_e]  +  y[c])   (self loop direct)
  h = relu(BN(agg))

Perf structure (HW-measured: the SWDGE indexed gather is ~95% of runtime,
~4.7ns/descriptor across 4 queues; everything else hides under it):
 - y is exchanged as TWO row-range tables via Shared-output AllGathers (the
   fast collective path), so the A-table gather/segment pass overlaps the
   B-table AllGather.
 - One dma_gather piece per few chunks of each (window, table) run; each
   core's padding is a trailing run of idx=-1 slots the ucode skips, with a
   shared num_idxs_reg equal to the cross-core max real count (dummy idx=0
   slots make the count identical on every core).
 - Self loops never gather: their y stays SBUF-resident node-major and is
   added with one ACT mul + DVE add per tile.
 - The segment-sum is a TensorE matmul of each gathered 128-edge chunk
   against a one-hot S matrix built on DVE (is_equal vs iota), accumulated in
   PSUM per 512-destination block; BN stats transposes run inside pass B so
   they overlap the gather DMA.
The instruction stream is identical on all 8 cores (SPMD); all per-core
variation lives in the input data (indices, selection metadata, padding).
"""

import numpy as np
import ml_dtypes

import concourse.bacc as bacc
import concourse.bass as bass
import concourse.tile as tile
from concourse import mybir
from concourse.bass_utils import run_bass_kernel_spmd

BF16 = ml_dtypes.bfloat16
P = 128          # partitions / chunk size
WIN = 64         # dest window width (S matrix width)
BLOCK_WINS = 8   # windows per PSUM block (8*64 = 512 dests)
PIECE_CHUNKS = 99  # chunks per gather piece; whole runs (~640 desc) pipeline best
EPS = 1e-5


class Dims:
    def __init__(self, N=50000, E=800000, F=96, H=128, G=64, C=50,
                 ncores=8):
        assert N % ncores == 0
        self.N, self.E, self.F, self.H = N, E, F, H
        self.G, self.C = G, C
        self.ncores = ncores
        self.shard = N // ncores
        self.ntile = ceil_div(self.shard, P)  # node tiles per shard
        self.shard_pad = self.ntile * P       # padded shard rows in y table
        # local row-range split: table A = rows [0, S0), table B = [S0, pad).
        # Both 512-aligned (mt-group granularity); each table's all-gathered
        # row count stays within int16 index range for dma_gather.
        self.S0 = 3072
        self.S1 = self.shard_pad - self.S0
        self.npadA = self.S0 * ncores         # 24576 < 32768
        self.npadB = self.S1 * ncores         # 25600 < 32768
        assert self.npadA < 32768 and self.npadB < 32768
        self.nwin = ceil_div(self.shard, WIN)
        self.nblk = ceil_div(self.nwin, BLOCK_WINS)


def ceil_div(a, b):
    return (a + b - 1) // b


# ----------------------------------------------------------------------------
# Host planning: pure index/graph preprocessing (functions of edge_index/batch)
# ----------------------------------------------------------------------------

class Plan:
    pass


def make_plan(d: Dims, edge_index: np.ndarray, batch: np.ndarray) -> Plan:
    pl = Plan()
    N, E = d.N, d.E
    # self-loops are NOT routed through the gather: each core adds
    # dinv^2 * y for its own nodes directly on-chip. deg still counts them.
    rows = edge_index[0].astype(np.int64)
    cols = edge_index[1].astype(np.int64)
    deg = (np.bincount(cols, minlength=N) + 1).astype(np.float64)
    dinv = (1.0 / np.sqrt(np.maximum(deg, 1.0))).astype(np.float32)

    # Degree-balanced node -> (core, slot) assignment: the shared gather
    # schedule pays max-over-cores edges per (window, table) run, so a snake
    # deal by descending in-degree makes each window's edge count nearly
    # equal across cores (the output [G, C] is permutation-invariant).
    indeg = np.bincount(cols, minlength=N)
    order = np.argsort(-indeg, kind="stable")
    rounds, lanes = np.divmod(np.arange(N), d.ncores)
    lanes = np.where(rounds % 2 == 0, lanes, d.ncores - 1 - lanes)
    asg_core = np.empty(N, np.int64)
    asg_slot = np.empty(N, np.int64)
    asg_core[order] = lanes
    # stride-scatter the degree-ranked rounds over slots (97 coprime to
    # shard) so each 64-slot window mixes ranks: window edge counts stay
    # near-equal both across cores and across windows
    asg_slot[order] = (rounds * 97) % d.shard
    inv = np.empty((d.ncores, d.shard), np.int64)
    inv[asg_core, asg_slot] = np.arange(N)
    pl.inv = inv

    core_of = asg_core[cols]
    # remap source node id to its position in table A or B of the split
    # all-gathered y tables (half 0 = table A, half 1 = table B); the stored
    # index is already table-local.
    k_src = asg_core[rows]
    r_loc = asg_slot[rows]
    lo_all = r_loc < d.S0
    pid = np.where(lo_all, k_src * d.S0 + r_loc,
                   k_src * d.S1 + (r_loc - d.S0))
    # Per (core, window, half) edge lists, edges sorted by local dest.
    dst_slot = asg_slot[cols]
    per_core = []
    for k in range(d.ncores):
        m = core_of == k
        r = pid[m]
        c = dst_slot[m]
        lo = lo_all[m]
        order = np.argsort(c, kind="stable")
        r, c, lo = r[order], c[order], lo[order]
        w = c // WIN
        lists = {}
        # bucket by (window, half) preserving dest order
        for half_id, mask in ((0, lo), (1, ~lo)):
            rw, cw, ww = r[mask], c[mask], w[mask]
            # indices where window changes
            for wi in range(d.nwin):
                sel = ww == wi
                lists[(wi, half_id)] = (rw[sel], cw[sel])
        per_core.append(lists)

    # Shared chunk schedule: R[w][half] = max over cores of ceil(count/128)
    R = np.zeros((d.nwin, 2), dtype=np.int64)
    for k in range(d.ncores):
        for (wi, hf), (rw, cw) in per_core[k].items():
            R[wi, hf] = max(R[wi, hf], ceil_div(len(rw), P))
    R = np.maximum(R, 0)
    # every window must be initialized in PSUM: ensure at least one chunk
    for wi in range(d.nwin):
        if R[wi].sum() == 0:
            R[wi, 0] = 1

    # Build the chunk stream: per block: [lo chunks (w asc)] ++ [hi chunks]
    stream = []          # list of (window, half) per chunk position
    groups = []          # (block, half, chunk_start, chunk_count)
    for b in range(d.nblk):
        wlo = b * BLOCK_WINS
        whi = min(wlo + BLOCK_WINS, d.nwin)
        for hf in (0, 1):
            g0 = len(stream)
            for wi in range(wlo, whi):
                for _ in range(R[wi, hf]):
                    stream.append((wi, hf))
            groups.append((b, hf, g0, len(stream) - g0))
    C_grid = len(stream)
    tot_slots = C_grid * P

    # start/stop flags: matmul start=True zeroes the ENTIRE 2KB PSUM strip of
    # its output partitions, so exactly one start per (block, parity strip) --
    # the first chunk in stream order touching that strip; stop on the last.
    # all-accumulate scheme: the block PSUM tile is DVE-memset to zero, every
    # matmul uses start=False (accumulate). A start=True would zero the whole
    # 2KB PSUM strip of its partitions, wiping sibling windows in the bank.
    start_flag = np.zeros(C_grid, dtype=bool)
    stop_flag = np.zeros(C_grid, dtype=bool)

    # Fill per-core slot data. Pad slots keep idx=-1: the gather ucode skips
    # trailing negative indices, so per-(window,half)-run gather instructions
    # transfer only each core's real edges (padding varies per core).
    idx_all = np.full((d.ncores, tot_slots), -1, dtype=np.int16)
    A_all = np.full((d.ncores, C_grid, P), 300.0, dtype=np.float32)
    # chunk positions per (window, half) in stream order:
    pos_of = {}
    for pos, key in enumerate(stream):
        pos_of.setdefault(key, []).append(pos)
    # shared per-run valid count V = max over cores of real edges in the run.
    # num_idxs_reg must equal the count of non-negative indices and is a
    # shared immediate, so every core pads its run with dummy (idx=0, no
    # dest) slots up to V; slots beyond V keep idx=-1 and are skipped.
    V = np.zeros((d.nwin, 2), dtype=np.int64)
    for k in range(d.ncores):
        for (wi, hf), (rw, cw) in per_core[k].items():
            V[wi, hf] = max(V[wi, hf], len(rw))
    for wi in range(d.nwin):
        if V[wi].sum() == 0:
            V[wi, 0] = 1      # matches the R fixup: run exists, 1 dummy slot
    for k in range(d.ncores):
        for (wi, hf), (rw, cw) in per_core[k].items():
            n = len(rw)
            positions = pos_of.get((wi, hf), [])
            if not positions:
                assert n == 0
                continue
            assert n <= len(positions) * P
            vals = rw          # already table-local (split tables A/B)
            crel = cw - wi * WIN
            for j, pos in enumerate(positions):
                a, bnd = j * P, min((j + 1) * P, n)
                if a < n:
                    cnt = bnd - a
                    idx_all[k, pos * P: pos * P + cnt] = \
                        vals[a:bnd].astype(np.int16)
                    A_all[k, pos, :cnt] = crel[a:bnd].astype(np.float32)
            # dummy-valid padding up to the shared count V
            run0 = positions[0] * P
            idx_all[k, run0 + n: run0 + V[wi, hf]] = 0

    # wrap idx to the [128, tot_slots//16] layout dma_gather wants:
    # slot i -> [16*c + i%16, i//16] for every q7 core c
    S16 = tot_slots // 16
    idx_wrapped = np.zeros((d.ncores, P, S16), dtype=np.int16)
    for k in range(d.ncores):
        w16 = idx_all[k].reshape(S16, 16).T  # [16, S16]
        idx_wrapped[k] = np.tile(w16, (8, 1))

    # A matrix in [128 partitions=slot%128, C_grid] layout
    A_pt = np.transpose(A_all, (0, 2, 1)).astype(BF16)  # [cores, 128, C_grid]

    # per-core node-major helper arrays
    dinv_pt = np.zeros((d.ncores, P, d.ntile), dtype=np.float32)
    pool_pt = np.zeros((d.ncores, P, d.ntile, d.G), dtype=np.float32)
    for k in range(d.ncores):
        for t in range(d.ntile):
            for p in range(P):
                n0 = t * P + p
                if n0 < d.shard:
                    node = inv[k, n0]
                    dinv_pt[k, p, t] = dinv[node]
                    pool_pt[k, p, t, batch[node]] = 1.0

    cnts = np.bincount(batch, minlength=d.G).astype(np.float32)
    inv_cnt = (1.0 / np.maximum(cnts, 1.0)).reshape(d.G, 1)

    pl.R, pl.stream, pl.groups, pl.V = R, stream, groups, V
    pl.C_grid, pl.tot_slots = C_grid, tot_slots
    pl.start_flag, pl.stop_flag = start_flag, stop_flag
    pl.idx_wrapped, pl.A_pt = idx_wrapped, A_pt
    pl.dinv_pt, pl.pool_pt, pl.inv_cnt = dinv_pt, pool_pt.reshape(d.ncores, P, -1), inv_cnt
    pl.max_lo_chunks = max(g[3] for g in groups if g[1] == 0)
    pl.max_hi_chunks = max(g[3] for g in groups if g[1] == 1)
    blk_tot = {}
    for b, hf, g0, gc in groups:
        blk_tot[b] = blk_tot.get(b, 0) + gc
    pl.max_blk_chunks = max(blk_tot.values())
    return pl


# ----------------------------------------------------------------------------
# Bass program
# ----------------------------------------------------------------------------

def build_program(d: Dims, pl: Plan, debug=False, repeat=1, ablate=()):
    nc = bacc.Bacc("TRN2", target_bir_lowering=False, debug=False,
                   num_devices=d.ncores, num_swdge_queues=4)
    f32, bf16, i16 = mybir.dt.float32, mybir.dt.bfloat16, mybir.dt.int16

    def din(name, shape, dt=f32):
        return nc.dram_tensor(name, shape, dt, kind="ExternalInput").ap()

    xt = din("xt", [d.F, d.shard])
    W1 = din("W1", [d.F, d.H])
    W2 = din("W2", [d.H, d.H], bf16)
    Wc = din("Wc", [d.H, d.C])
    g1 = din("g1", [d.H, 1])
    be1 = din("be1", [d.H, 1])
    g2 = din("g2", [d.H, 1])
    be2 = din("be2", [d.H, 1])
    idx_d = din("idx", [P, pl.tot_slots // 16], i16)
    if "gathpair" in ablate or "gathhalf" in ablate:
        idxh_d = din("idxh", [P, pl.tot_slots // 32], i16)
    A_d = din("A", [P, pl.C_grid], bf16)
    dinv_d = din("dinv_pt", [P, d.ntile])
    pool_d = din("pool_pt", [P, d.ntile * d.G])
    invc_d = din("inv_cnt", [d.G, 1])
    bcr_d = din("bc_rep", [d.G, d.C])
    iota_d = din("iota", [P, WIN], bf16)
    ident_d = din("ident", [P, P])
    out_d = nc.dram_tensor("out", [d.G, d.C], f32, kind="ExternalOutput").ap()
    if debug:
        dbg_agg = nc.dram_tensor("dbg_agg", [P, d.ntile * d.H], f32,
                                 kind="ExternalOutput").ap()
        dbg_h = nc.dram_tensor("dbg_h", [d.H, d.ntile * P], f32,
                               kind="ExternalOutput").ap()
        dbg_y = nc.dram_tensor("dbg_y", [d.npadA + d.npadB, d.H], f32,
                               kind="ExternalOutput").ap()

    rg = [list(range(d.ncores))]

    with tile.TileContext(nc) as tc:
        with (
            tc.tile_pool(name="const", bufs=1) as cpool,
            tc.tile_pool(name="work", bufs=2) as wpool,
            tc.tile_pool(name="glo", bufs=4) as gpool_lo,
            tc.tile_pool(name="ghi", bufs=4) as gpool_hi,
            tc.tile_pool(name="spool", bufs=3) as spool,
            tc.tile_pool(name="big", bufs=1) as bigpool,
            tc.tile_pool(name="pseg", bufs=3, space="PSUM") as pseg,
            tc.tile_pool(name="pmm", bufs=2, space="PSUM") as pmm,
            tc.tile_pool(name="ptr", bufs=3, space="PSUM") as ptr,
            tc.tile_pool(name="dram", bufs=1, space="DRAM") as dpool,
            tc.tile_pool(name="dram_y", bufs=2, space="DRAM") as ypool,
            tc.tile_pool(name="dram_so", bufs=2, space="DRAM") as sopool,
        ):
            # ---- load constants ----
            def cload(ap, shape, dt=f32, name=None):
                t = cpool.tile(shape, dt, tag=name)
                nc.sync.dma_start(out=t[:], in_=ap)
                return t

            W1_s = cload(W1[:], [d.F, d.H], name="W1")
            W2_s = cload(W2[:], [d.H, d.H], bf16, name="W2")
            Wc_s = cload(Wc[:], [d.H, d.C], name="Wc")
            g1_s = cload(g1[:], [d.H, 1], name="g1")
            be1_s = cload(be1[:], [d.H, 1], name="be1")
            g2_s = cload(g2[:], [d.H, 1], name="g2")
            be2_s = cload(be2[:], [d.H, 1], name="be2")
            idx_s = cload(idx_d[:], [P, pl.tot_slots // 16], i16, name="idx")
            if "gathpair" in ablate or "gathhalf" in ablate:
                idxh_s = cload(idxh_d[:], [P, pl.tot_slots // 32], i16,
                               name="idxh")
            A_s = cload(A_d[:], [P, pl.C_grid], bf16, name="A")
            dinv_s = cload(dinv_d[:], [P, d.ntile], name="dinv")
            pool_s = cload(pool_d[:], [P, d.ntile * d.G], name="pool")
            invc_s = cload(invc_d[:], [d.G, 1], name="invc")
            bcr_s = cload(bcr_d[:], [d.G, d.C], name="bcr")
            iota_s = cload(iota_d[:], [P, WIN], bf16, name="iota")
            ident_s = cload(ident_d[:], [P, P], name="ident")

            # pool matrix as bf16 for matmul
            pool_bf = cpool.tile([P, d.ntile * d.G], bf16, tag="poolbf")
            nc.vector.tensor_copy(out=pool_bf[:], in_=pool_s[:])

            eps_s = cpool.tile([d.H, 1], f32, tag="eps")
            nc.vector.memset(eps_s[:], EPS)
            ident_bf = cpool.tile([P, P], bf16, tag="identbf")
            nc.vector.tensor_copy(out=ident_bf[:], in_=ident_s[:])

            # ---- internal DRAM for collectives ----
            adsp = "Local" if "nosharedout" in ablate else "Shared"
            y_own = dpool.tile([d.shard_pad, d.H], bf16)
            stats_in = dpool.tile([d.H, 2], f32)
            pool_in = dpool.tile([d.G, d.H], f32)
            pool_out = dpool.tile([d.G, d.H], f32, addr_space=adsp)

            h_fm = None  # feature-major relu'd activations [H, shard]
            gq = [0]
            qload = [0, 0, 0, 0]  # greedy per-queue descriptor balance

            # one-time zero of the gather pool buffers: slots skipped by the
            # ucode (trailing idx=-1) leave SBUF untouched, and S=0 only
            # protects against finite garbage (0*NaN would poison PSUM)
            for gp, mg, tg in ((gpool_lo, pl.max_lo_chunks, "g0"),
                               (gpool_hi, pl.max_hi_chunks, "g1")):
                for _ in range(4):  # must touch every pool buffer
                    zt = gp.tile([P, mg, d.H], bf16, tag=tg, name=f"z{tg}")
                    nc.vector.memset(zt[:], 0.0)

            for rep in range(repeat):
              for layer in range(2):
                y_fullA = ypool.tile([d.npadA, d.H], bf16, addr_space=adsp,
                                     name=f"y_fullA_r{rep}l{layer}")
                y_fullB = ypool.tile([d.npadB, d.H], bf16, addr_space=adsp,
                                     name=f"y_fullB_r{rep}l{layer}")
                stats_out = sopool.tile([d.H, 2], f32, addr_space=adsp,
                                        name=f"stats_out_r{rep}l{layer}")
                # ---------- y = dinv * (X @ W)  (own shard, node-major) ----
                # staged: compute rows [0,S0) then AllGather table A, then
                # rows [S0,shard) and AllGather table B, so the A-pass
                # gather/segmm below overlaps the B AllGather. The node-major
                # y stays resident in SBUF for the self-loop contribution.
                y_sb = bigpool.tile([P, d.ntile, d.H], bf16, tag="y_sb")
                n_mt = ceil_div(d.shard, 512)
                for phase in (0, 1):
                    mtr = (range(0, d.S0 // 512) if phase == 0
                           else range(d.S0 // 512, n_mt))
                    for mt in mtr:
                        c0 = mt * 512
                        cw = min(512, d.shard - c0)
                        nst = ceil_div(cw, P)
                        if layer == 0:
                            rhs_t = wpool.tile([d.F, 512], f32, tag="xt_t")
                            nc.sync.dma_start(out=rhs_t[:, :cw],
                                              in_=xt[:, c0:c0 + cw])
                            lhsT, rhs_ap = W1_s[:, :], rhs_t[:, :cw]
                        else:
                            lhsT, rhs_ap = W2_s[:, :], h_fm[:, c0:c0 + cw]
                        xw_ps = pmm.tile([d.H, 512], f32, tag="xw")
                        nc.tensor.matmul(out=xw_ps[:, :cw], lhsT=lhsT,
                                         rhs=rhs_ap, start=True, stop=True)
                        xw_sb = wpool.tile([d.H, 512], f32, tag="xw_sb")
                        nc.scalar.copy(out=xw_sb[:, :cw], in_=xw_ps[:, :cw])
                        # transpose 128-node subtiles; dinv scale in ACT evac
                        for st in range(nst):
                            t_global = mt * 4 + st
                            n0 = st * P
                            nw = min(P, cw - n0)
                            tr_ps = ptr.tile([P, d.H], f32, tag="ptr")
                            nc.tensor.transpose(out=tr_ps[:nw, :],
                                                in_=xw_sb[:, n0:n0 + nw],
                                                identity=ident_s[:])
                            nc.scalar.mul(out=y_sb[:nw, t_global, :],
                                          in_=tr_ps[:nw, :],
                                          mul=dinv_s[:nw,
                                                     t_global:t_global + 1])
                        nc.sync.dma_start(
                            out=y_own[c0:c0 + nst * P, :].rearrange(
                                "(t p) f -> p t f", p=P),
                            in_=y_sb[:, mt * 4:mt * 4 + nst, :])
                    y_in = (y_own[0:d.S0, :] if phase == 0
                            else y_own[d.S0:d.shard_pad, :])
                    y_out = y_fullA if phase == 0 else y_fullB
                    if "nogather_collective" in ablate:
                        nc.sync.dma_start(
                            out=y_out[0:(d.S0 if phase == 0 else d.S1), :],
                            in_=y_in)
                    else:
                        nc.gpsimd.collective_compute(
                            "AllGather", mybir.AluOpType.bypass,
                            replica_groups=rg,
                            ins=[y_in.opt()], outs=[y_out.opt()])

                # ---------- gather + segment matmul: pass A, then pass B ----
                probe = ("gathpair" in ablate) or ("gathhalf" in ablate)
                agg_dm = bigpool.tile([P, d.ntile, d.H], bf16, tag="agg_dm")
                # feature-major bf16 copy of agg + BN stats, produced
                # incrementally during pass B (overlaps gather DMA)
                agg_fm = bigpool.tile([d.H, d.ntile * P], bf16, tag="agg_fm")
                s1p = wpool.tile([d.H, d.ntile], f32, tag="s1p")
                s2p = wpool.tile([d.H, d.ntile], f32, tag="s2p")
                scratch = wpool.tile([d.H, P], f32, tag="scr")
                if d.shard % P:
                    nc.vector.memset(agg_dm[:, d.ntile - 1, :], 0.0)
                for hf in (0, 1):
                    gpool = gpool_lo if hf == 0 else gpool_hi
                    ysrc = (y_fullA if hf == 0 else y_fullB)[:, :]
                    mgc = pl.max_lo_chunks if hf == 0 else pl.max_hi_chunks
                    for b in range(d.nblk):
                        wlo = b * BLOCK_WINS
                        whi = min(wlo + BLOCK_WINS, d.nwin)
                        _, _, g0, gcnt = pl.groups[2 * b + hf]
                        empty_b = hf == 1 and gcnt == 0
                        blk_ps = None
                        if not empty_b:
                            blk_ps = pseg.tile([P, 4 * d.H], f32, tag="seg")
                            nc.vector.memset(blk_ps[:], 0.0)
                        gt = None
                        if (gcnt and not empty_b
                                and "nodmagather" not in ablate and not probe):
                            gt = gpool.tile([P, mgc, d.H], bf16, tag=f"g{hf}")
                            if "contiggather" in ablate:
                                nc.sync.dma_start(
                                    out=gt[:, :gcnt, :],
                                    in_=ysrc[0:gcnt * P, :].rearrange(
                                        "(s p) f -> p s f", p=P))
                            else:
                                # one gather per window run: each core's
                                # padding is a trailing run of idx=-1 slots,
                                # which the ucode skips (no transfer)
                                roff = 0
                                for wi in range(wlo, whi):
                                    rc = int(pl.R[wi, hf])
                                    if rc == 0:
                                        continue
                                    V_run = (rc * P if PAD0
                                             else int(pl.V[wi, hf]))
                                    # split the run into small pieces across
                                    # queues; valid slots are a prefix of the
                                    # run, so each piece's reg count is exact
                                    # and empty pieces are skipped entirely
                                    pc = 0
                                    while pc < rc:
                                        pcw = min(PIECE_CHUNKS, rc - pc)
                                        reg = max(0, min(V_run - pc * P,
                                                         pcw * P))
                                        if reg > 0:
                                            ns_pp = pcw * P
                                            s0 = (g0 + roff + pc) * P
                                            qn = gq[0] % 4
                                            nc.gpsimd.dma_gather(
                                                out_ap=gt[:, roff + pc:
                                                          roff + pc + pcw, :],
                                                in_ap=ysrc,
                                                idxs_ap=idx_s[
                                                    :, s0 // 16:
                                                    (s0 + ns_pp) // 16],
                                                num_idxs=ns_pp,
                                                num_idxs_reg=reg,
                                                elem_size=d.H,
                                                single_packet=False,
                                                queue_num=qn,
                                            )
                                            gq[0] += 1
                                        pc += pcw
                                    roff += rc
                                assert roff == gcnt
                        elif gcnt and probe:
                            # timing probes: same bytes/half bytes with half
                            # the descriptors; gathered data unused (implies
                            # no segmm matmuls for this pass)
                            pair = "gathpair" in ablate
                            esz = 2 * d.H if pair else d.H
                            gtp = gpool.tile([P, ceil_div(mgc, 2), esz], bf16,
                                             tag=f"gp{hf}")
                            ns_p = gcnt * P
                            ns_h = ns_p // 2
                            ysrc_p = (ysrc.rearrange("(a two) f -> a (two f)",
                                                     two=2) if pair else ysrc)
                            nc.gpsimd.dma_gather(
                                out_ap=gtp[:, 0:ceil_div(ns_h, P), :],
                                in_ap=ysrc_p,
                                idxs_ap=idxh_s[:, (g0 * P) // 32:
                                               (g0 * P) // 32 + ns_h // 16],
                                num_idxs=ns_h,
                                num_idxs_reg=ns_h,
                                elem_size=esz,
                                single_packet=False,
                                queue_num=gq[0] % 4,
                            )
                            gq[0] += 1
                        if gcnt and gt is not None:
                            S_t = spool.tile(
                                [P, max(pl.max_lo_chunks, pl.max_hi_chunks),
                                 WIN], bf16, tag="S")
                            a_b = A_s[:, g0:g0 + gcnt].unsqueeze(2) \
                                .broadcast_to([P, gcnt, WIN])
                            i_b = iota_s[:].unsqueeze(1) \
                                .broadcast_to([P, gcnt, WIN])
                            nc.vector.tensor_tensor(out=S_t[:, :gcnt, :],
                                                    in0=a_b, in1=i_b,
                                                    op=mybir.AluOpType.is_equal)
                            for pos in (() if "nosegmm" in ablate
                                        else range(g0, g0 + gcnt)):
                                wi, _hx = pl.stream[pos]
                                lc = pos - g0
                                w_in_b = wi - wlo
                                wpp = P // WIN
                                pof = WIN * (w_in_b % wpp)
                                fof = d.H * (w_in_b // wpp)
                                nc.tensor.matmul(
                                    out=blk_ps[pof:pof + WIN, fof:fof + d.H],
                                    lhsT=S_t[:, lc, :],
                                    rhs=gt[:, lc, :],
                                    start=False, stop=False,
                                    skip_group_check=True,
                                )
                        # evacuate: dest-major agg with dinv scaling; pass B
                        # accumulates on top of pass A, then immediately
                        # transposes each finished tile for BN stats + the
                        # feature-major agg copy (overlaps later gathers)
                        for w4 in range(ceil_div((whi - wlo) * WIN, P)):
                            t_global = (BLOCK_WINS * WIN // P) * b + w4
                            nw = min(P, d.shard - t_global * P)
                            if hf == 0:
                                nc.scalar.mul(
                                    out=agg_dm[:nw, t_global, :],
                                    in_=blk_ps[:nw, w4 * d.H:(w4 + 1) * d.H],
                                    mul=dinv_s[:nw, t_global:t_global + 1])
                                # direct self-loop term: dinv^2 * xw = dinv * y
                                # (y already carries one dinv factor)
                                slt = wpool.tile([P, d.H], bf16, tag="slt")
                                nc.scalar.mul(
                                    out=slt[:nw, :],
                                    in_=y_sb[:nw, t_global, :],
                                    mul=dinv_s[:nw, t_global:t_global + 1])
                                nc.vector.tensor_tensor(
                                    out=agg_dm[:nw, t_global, :],
                                    in0=agg_dm[:nw, t_global, :],
                                    in1=slt[:nw, :],
                                    op=mybir.AluOpType.add)
                                continue
                            if not empty_b:
                                evB = wpool.tile([P, d.H], bf16, tag="evB")
                                nc.scalar.mul(
                                    out=evB[:nw, :],
                                    in_=blk_ps[:nw, w4 * d.H:(w4 + 1) * d.H],
                                    mul=dinv_s[:nw, t_global:t_global + 1])
                                nc.vector.tensor_tensor(
                                    out=agg_dm[:nw, t_global, :],
                                    in0=agg_dm[:nw, t_global, :],
                                    in1=evB[:nw, :],
                                    op=mybir.AluOpType.add)
                            t = t_global
                            tr_ps = ptr.tile([d.H, P], bf16, tag="ptr")
                            nc.tensor.transpose(out=tr_ps[:, :],
                                                in_=agg_dm[:, t, :],
                                                identity=ident_bf[:])
                            nc.scalar.activation(
                                out=agg_fm[:, t * P:(t + 1) * P],
                                in_=tr_ps[:],
                                func=mybir.ActivationFunctionType.Copy,
                                accum_out=s1p[:, t:t + 1])
                            nc.scalar.activation(
                                out=scratch[:], in_=tr_ps[:],
                                func=mybir.ActivationFunctionType.Square,
                                accum_out=s2p[:, t:t + 1])

                stats_sb = wpool.tile([d.H, 2], f32, tag="stats")
                nc.vector.tensor_reduce(out=stats_sb[:, 0:1], in_=s1p[:],
                                        axis=mybir.AxisListType.X,
                                        op=mybir.AluOpType.add)
                nc.vector.tensor_reduce(out=stats_sb[:, 1:2], in_=s2p[:],
                                        axis=mybir.AxisListType.X,
                                        op=mybir.AluOpType.add)
                nc.sync.dma_start(out=stats_in[:], in_=stats_sb[:])
                if "nostatsar" in ablate:
                    nc.sync.dma_start(out=stats_out[:], in_=stats_in[:])
                else:
                    nc.gpsimd.collective_compute(
                        "AllReduce", mybir.AluOpType.add, replica_groups=rg,
                        ins=[stats_in.opt()], outs=[stats_out.opt()])
                stats_g = wpool.tile([d.H, 2], f32, tag="statsg")
                nc.sync.dma_start(out=stats_g[:], in_=stats_out[:])
                # mean/var -> scale/bias
                mv = wpool.tile([d.H, 6], f32, tag="mv")
                inv_n = 1.0 / d.N
                nc.vector.tensor_scalar(out=mv[:, 0:1], in0=stats_g[:, 0:1],
                                        scalar1=inv_n, scalar2=None,
                                        op0=mybir.AluOpType.mult)  # mean
                nc.vector.tensor_scalar(out=mv[:, 1:2], in0=stats_g[:, 1:2],
                                        scalar1=inv_n, scalar2=None,
                                        op0=mybir.AluOpType.mult)  # E[x^2]
                nc.vector.tensor_tensor(out=mv[:, 2:3], in0=mv[:, 0:1],
                                        in1=mv[:, 0:1],
                                        op=mybir.AluOpType.mult)   # mean^2
                nc.vector.tensor_tensor(out=mv[:, 2:3], in0=mv[:, 1:2],
                                        in1=mv[:, 2:3],
                                        op=mybir.AluOpType.subtract)  # var
                nc.scalar.activation(out=mv[:, 3:4], in_=mv[:, 2:3],
                                     func=mybir.ActivationFunctionType.Sqrt,
                                     bias=eps_s[:])                # std
                nc.vector.reciprocal(out=mv[:, 4:5], in_=mv[:, 3:4])
                gg = g1_s if layer == 0 else g2_s
                bb = be1_s if layer == 0 else be2_s
                nc.vector.tensor_tensor(out=mv[:, 4:5], in0=mv[:, 4:5],
                                        in1=gg[:], op=mybir.AluOpType.mult)
                # bias = be - mean*scale
                nc.vector.tensor_tensor(out=mv[:, 5:6], in0=mv[:, 0:1],
                                        in1=mv[:, 4:5],
                                        op=mybir.AluOpType.mult)
                nc.vector.tensor_tensor(out=mv[:, 5:6], in0=bb[:],
                                        in1=mv[:, 5:6],
                                        op=mybir.AluOpType.subtract)
                if debug and layer == 0:
                    for tbl, base in ((y_fullA, 0), (y_fullB, d.npadA)):
                        for t in range(tbl.shape[0] // P):
                            dbg_y_bf = wpool.tile([P, d.H], bf16,
                                                  tag="dbgybf")
                            dbg_y_sb = wpool.tile([P, d.H], f32, tag="dbgy")
                            nc.sync.dma_start(
                                out=dbg_y_bf[:],
                                in_=tbl[t * P:(t + 1) * P, :])
                            nc.vector.tensor_copy(out=dbg_y_sb[:],
                                                  in_=dbg_y_bf[:])
                            nc.sync.dma_start(
                                out=dbg_y[base + t * P:base + (t + 1) * P, :],
                                in_=dbg_y_sb[:])
                    nc.sync.dma_start(
                        out=dbg_agg[:],
                        in_=agg_dm[:].rearrange("p t f -> p (t f)"))
                h_fm = bigpool.tile([d.H, d.ntile * P], bf16, tag="h_fm")
                for t in range(d.ntile):
                    nc.scalar.activation(out=h_fm[:, t * P:(t + 1) * P],
                                         in_=agg_fm[:, t * P:(t + 1) * P],
                                         func=mybir.ActivationFunctionType.Relu,
                                         scale=mv[:, 4:5], bias=mv[:, 5:6])

            if debug:
                dbg_h_sb = wpool.tile([d.H, d.ntile * P], f32, tag="dbgh")
                nc.vector.tensor_copy(out=dbg_h_sb[:], in_=h_fm[:])
                nc.sync.dma_start(out=dbg_h[:], in_=dbg_h_sb[:])
            # ---------- pooling ----------
            # node-major h2 tiles via transpose, then matmul with pool matrix
            pool_ps = ptr.tile([d.G, d.H], f32, tag="ptr")
            for t in range(d.ntile):
                tr_ps = ptr.tile([P, d.H], bf16, tag="ptr")
                nc.tensor.transpose(out=tr_ps[:, :],
                                    in_=h_fm[:, t * P:(t + 1) * P],
                                    identity=ident_bf[:])
                h_dm = wpool.tile([P, d.H], bf16, tag="h_dm")
                nc.scalar.copy(out=h_dm[:], in_=tr_ps[:])
                nc.tensor.matmul(
                    out=pool_ps[:, :],
                    lhsT=pool_bf[:, t * d.G:(t + 1) * d.G],
                    rhs=h_dm[:],
                    start=(t == 0), stop=(t == d.ntile - 1))
            pool_sb = wpool.tile([d.G, d.H], f32, tag="poolsb")
            nc.vector.tensor_scalar(out=pool_sb[:], in0=pool_ps[:],
                                    scalar1=invc_s[:], scalar2=None,
                                    op0=mybir.AluOpType.mult)
            nc.sync.dma_start(out=pool_in[:], in_=pool_sb[:])
            if "nopoolar" in ablate:
                nc.sync.dma_start(out=pool_out[:], in_=pool_in[:])
            else:
                nc.gpsimd.collective_compute(
                    "AllReduce", mybir.AluOpType.add, replica_groups=rg,
                    ins=[pool_in.opt()], outs=[pool_out.opt()])
            pooled = wpool.tile([d.G, d.H], f32, tag="pooled")
            nc.sync.dma_start(out=pooled[:], in_=pool_out[:])
            # transpose pooled -> [H, G]
            pooled_t_ps = ptr.tile([d.H, d.G], f32, tag="ptr")
            nc.tensor.transpose(out=pooled_t_ps[:, :], in_=pooled[:],
                                identity=ident_s[:d.G, :d.G])
            pooled_t = wpool.tile([d.H, d.G], f32, tag="pooledtsb")
            nc.scalar.copy(out=pooled_t[:], in_=pooled_t_ps[:])
            out_ps = ptr.tile([d.G, d.C], f32, tag="ptr")
            nc.tensor.matmul(out=out_ps[:], lhsT=pooled_t[:], rhs=Wc_s[:],
                             start=True, stop=True)
            out_sb = wpool.tile([d.G, d.C], f32, tag="outsb")
            nc.vector.tensor_tensor(out=out_sb[:], in0=out_ps[:],
                                    in1=bcr_s[:], op=mybir.AluOpType.add)
            nc.scalar.activation(out=out_sb[:], in_=out_sb[:],
                                 func=mybir.ActivationFunctionType.Sigmoid)
            nc.sync.dma_start(out=out_d[:], in_=out_sb[:])

    nc.compile()
    return nc


# ----------------------------------------------------------------------------
# Entry point
# ----------------------------------------------------------------------------

def make_in_maps(d: Dims, pl: Plan, inputs):
    x = np.asarray(inputs["x"], np.float32)
    W1 = np.asarray(inputs["W1"], np.float32)
    W2 = np.asarray(inputs["W2"], np.float32)
    Wc = np.asarray(inputs["Wc"], np.float32)
    g1 = np.asarray(inputs["g1"], np.float32).reshape(d.H, 1)
    be1 = np.asarray(inputs["be1"], np.float32).reshape(d.H, 1)
    g2 = np.asarray(inputs["g2"], np.float32).reshape(d.H, 1)
    be2 = np.asarray(inputs["be2"], np.float32).reshape(d.H, 1)
    bc = np.asarray(inputs["bc"], np.float32)
    xt = np.ascontiguousarray(x.T)
    iota = np.tile(np.arange(WIN, dtype=np.float32), (P, 1)).astype(BF16)
    ident = np.eye(P, dtype=np.float32)
    bc_rep = np.tile(bc.reshape(1, d.C), (d.G, 1)).astype(np.float32)
    in_maps = []
    for k in range(d.ncores):
        in_maps.append({
            "xt": np.ascontiguousarray(xt[:, pl.inv[k]]),
            "W1": W1, "W2": W2.astype(BF16), "Wc": Wc,
            "g1": g1, "be1": be1, "g2": g2, "be2": be2,
            "idx": (np.maximum(pl.idx_wrapped[k], 0) if PAD0
                    else pl.idx_wrapped[k]),
            "idxh": np.maximum(pl.idx_wrapped[k][:, :pl.tot_slots // 32], 0)
                    // 2,
            "A": np.ascontiguousarray(pl.A_pt[k]),
            "dinv_pt": pl.dinv_pt[k],
            "pool_pt": pl.pool_pt[k],
            "inv_cnt": pl.inv_cnt,
            "bc_rep": bc_rep,
            "iota": iota,
            "ident": ident,
        })
    return in_maps


PAD0 = False  # True: pad slots gather row 0 instead of being skipped (-1)


def kernel(**inputs) -> np.ndarray:
    d = Dims()
    edge_index = np.asarray(inputs["edge_index"], np.int64)
    batch = np.asarray(inputs["batch"], np.int64)
    pl = make_plan(d, edge_index, batch)
    nc = build_program(d, pl)
    in_maps = make_in_maps(d, pl, inputs)
    res = run_bass_kernel_spmd(nc, in_maps, core_ids=list(range(d.ncores)))
    return np.asarray(res.results[0]["out"], np.float32)

